# revision 1
# baseline (speedup 1.0000x reference)
"""Polynomial features (degree 2) + linear layer, distributed over 8 TRN2 cores.

reference: A = [x, {x_i*x_j for i<=j}] (8384 coeffs); out = A @ W.T + b.

Device algorithm (per core, batch shard 4096, feature-on-partition layout):
  - pairs are enumerated by circular distance class s in 0..64:
      class s, lane p  ->  unordered pair {p, (p+s) % 128}
    (each unordered pair appears exactly once; s=64 lanes >=64 are dups
    with zeroed weights)
  - host ships 16 rotated copies of x^T (rot d: row p = feature (p+d)%128)
    for d in D = {0..8, 16, 24, 32, 40, 48, 56, 64}; every class s is one
    bf16 DVE tensor_mul of two rotations with b - a = s (the hardware only
    allows 32-aligned partition bases, so all ops are full 128-partition,
    base 0 - the rotations do the shifting)
  - 66 matmuls (1 linear chunk + 65 class chunks, K=128 each) accumulate
    into PSUM [64 outs, 512 batch]; W is permuted host-side to match;
    bias is added in the PSUM->SBUF copy (DVE tensor_scalar_add)
  - TPB instructions have a single sync-wait slot, but Tile emits multiple
    waits on slot-recycling instructions; _split_multiwaits() post-processes
    the BIR, hoisting extra waits onto injected same-engine NOPs
"""

import numpy as np
import ml_dtypes

INPUT_DIM = 128
OUTPUT_DIM = 64
BATCH = 32768
N_CORES = 8
B_CORE = BATCH // N_CORES  # 4096
TILE_B = 512
N_TILES = B_CORE // TILE_B  # 8

ROT_SET = [0, 1, 2, 3, 4, 5, 6, 7, 8, 16, 24, 32, 40, 48, 56, 64]
N_ROT = len(ROT_SET)
ROT_IDX = {d: i for i, d in enumerate(ROT_SET)}

import os

GPS_OP_IDS = tuple(
    int(v) for v in os.environ.get("K_GPS_OPS", "").split(",") if v != ""
)


def _class_ops():
    """(a, b) rotation pair per distance class s=0..64 with b - a = s."""
    ops = []
    for s in range(65):
        if s <= 8:
            a, b = 0, s
        else:
            k = (s - 1) // 8  # 1..7
            anchor = 8 * k + 8
            a, b = anchor - s, anchor
        assert a in ROT_SET and b in ROT_SET and b - a == s, (s, a, b)
        ops.append((a, b))
    return ops


CLASS_OPS = _class_ops()


def _build_device_weights(W, b):
    """Permute W [64, 8384] into the device K-block layout.

    Returns w_packed [128, 66*64]: block j (j=0 linear, j=1+s class s)
    lives at free columns [j*64, (j+1)*64), partition p = K row p.
    Class s row p -> pair {p, (p+s)%128}; s=64 rows p>=64 are zeroed dups.
    """
    W = np.asarray(W, np.float32)
    n = INPUT_DIM
    pair_off = {}
    c = 0
    for i in range(n):
        for j in range(i, n):
            pair_off[(i, j)] = c
            c += 1
    assert c == 8256

    Wd = np.zeros((66, 128, OUTPUT_DIM), np.float32)
    Wd[0] = W[:, 0:128].T  # linear block
    seen = set()
    for s in range(65):
        a, _bb = CLASS_OPS[s]
        for p in range(128):
            u = (p + a) % 128
            v = (p + a + s) % 128
            i, j = (u, v) if u <= v else (v, u)
            if (i, j) in seen:
                continue  # duplicate lane (s=64 second half)
            seen.add((i, j))
            Wd[1 + s, p] = W[:, 128 + pair_off[(i, j)]]
    assert len(seen) == 8256, len(seen)
    w_packed = np.ascontiguousarray(
        Wd.transpose(1, 0, 2).reshape(128, 66 * OUTPUT_DIM)
    ).astype(ml_dtypes.bfloat16)
    return w_packed, np.asarray(b, np.float32)


def _split_multiwaits(nc, mybir):
    """TPB instructions have one sync-wait slot; hoist extras onto NOPs."""
    import bass_rust

    n_split = 0
    for fn in nc.m.functions:
        for bb in fn.blocks:
            out = []
            changed = False
            for inst in bb.instructions:
                si = getattr(inst, "sync_info", None)
                if si is not None and si.on_wait and len(si.on_wait) > 1:
                    for w in si.on_wait[:-1]:
                        n_split += 1
                        nop = bass_rust.InstNoOp(
                            name=f"I-mw{n_split}",
                            engine=inst.engine,
                            ins=[],
                            outs=[],
                            sync_info=mybir.SyncInfo(on_wait=[w], on_update=[]),
                            bass_nofuse=True,
                        )
                        out.append(nop)
                    inst.sync_info = mybir.SyncInfo(
                        on_wait=[si.on_wait[-1]], on_update=si.on_update
                    )
                    changed = True
                out.append(inst)
            if changed:
                bb.instructions = out
    return n_split


def build(x, W, b):
    """Build the Bass graph and per-core input maps. Returns (nc, in_maps)."""
    import concourse.bass as bass
    import concourse.mybir as mybir
    from concourse import tile

    bf16 = mybir.dt.bfloat16
    f32 = mybir.dt.float32

    # ---- host preprocessing ----
    xT = np.ascontiguousarray(np.asarray(x, np.float32).T).astype(
        ml_dtypes.bfloat16
    )  # [128, 32768]
    # xall[p, i, n] = feature (p + ROT_SET[i]) % 128 of sample n
    xall = np.stack([np.roll(xT, -d, axis=0) for d in ROT_SET], axis=1)
    w_packed, bias = _build_device_weights(W, b)

    # ---- device graph ----
    nc = bass.Bass()
    x_in = nc.declare_dram_parameter(
        "xall", [N_TILES, 128, N_ROT, TILE_B], bf16, isOutput=False
    )
    w_in = nc.declare_dram_parameter("Wd", [128, 66 * 64], bf16, isOutput=False)
    b_in = nc.declare_dram_parameter("bias", [OUTPUT_DIM, 1], f32, isOutput=False)
    out_ext = nc.declare_dram_parameter(
        "outT", [OUTPUT_DIM, B_CORE], f32, isOutput=True
    )

    # multi-class ops: one per anchor family, constant-stride rotation APs:
    # op 0 = classes 0..8 (rot0 x rot 0..8), ops 1..7 = classes 8k+1..8k+8
    MC_OPS = [list(range(0, 9))] + [
        list(range(8 * k + 1, 8 * k + 9)) for k in range(1, 8)
    ]
    GPS_OPS = set(GPS_OP_IDS)  # op indices computed on GpSimd

    def rot_group_ap(xrt, classes):
        """[128, len(classes), TILE_B] APs (in0, in1)."""
        m = len(classes)
        us = [ROT_IDX[CLASS_OPS[s][0]] for s in classes]
        vs = [ROT_IDX[CLASS_OPS[s][1]] for s in classes]

        def mk(idx):
            if all(i == idx[0] for i in idx):
                return xrt[:, idx[0] : idx[0] + 1, :].to_broadcast(
                    [128, m, TILE_B]
                )
            d = idx[1] - idx[0]
            assert all(idx[j + 1] - idx[j] == d for j in range(m - 1)), idx
            return xrt[:, idx[0] :: d, :][:, 0:m, :]

        return mk(us), mk(vs)

    with tile.TileContext(nc) as tc:
        with (
            tc.tile_pool(name="consts", bufs=1) as consts,
            tc.tile_pool(name="xc", bufs=3) as xcp,
            tc.tile_pool(name="prod", bufs=4) as prodp,
            tc.tile_pool(name="prodg", bufs=5) as prodgp,
            tc.tile_pool(name="outp", bufs=3) as outp,
            tc.tile_pool(name="psum", bufs=2, space="PSUM") as psump,
        ):
            w_sb = consts.tile([128, 66 * 64], bf16)
            nc.sync.dma_start(w_sb[:], w_in[:])
            b_sb = consts.tile([OUTPUT_DIM, 1], f32)
            nc.sync.dma_start(b_sb[:], b_in[:])

            xc_tiles = [None] * (N_TILES + 2)

            def load_xc(t):
                if t >= N_TILES:
                    return
                xt = xcp.tile([128, N_ROT, TILE_B], bf16, tag="xc", name="xc_t")
                nc.sync.dma_start(xt[:], x_in[t][:])
                xc_tiles[t] = xt

            load_xc(0)
            load_xc(1)
            for t in range(N_TILES):
                load_xc(t + 2)
                xrt = xc_tiles[t]

                # acc halves: even classes + linear -> partitions 0:64
                # (array cols 0-63), odd classes -> partitions 64:128
                acc = psump.tile([128, TILE_B], f32, name="acc")
                nc.tensor.matmul(
                    acc[0:64, :],
                    w_sb[:, 0:64],
                    xrt[:, 0, :],
                    start=True,
                    stop=False,
                    tile_position=(0, 0),
                )
                first_odd = True
                for k, classes in enumerate(MC_OPS):
                    m = len(classes)
                    pool_k = prodgp if k in GPS_OPS else prodp
                    tag = ("prodg" if k in GPS_OPS else "prod") + str(m)
                    p_t = pool_k.tile(
                        [128, m, TILE_B], bf16, tag=tag, name="p_t"
                    )
                    in0, in1 = rot_group_ap(xrt, classes)
                    eng = nc.gpsimd if k in GPS_OPS else nc.vector
                    eng.tensor_mul(p_t[:], in0, in1)
                    views = [
                        (s, p_t[:, j, :]) for j, s in enumerate(classes)
                    ]
                    for s, rhs in views:
                        half = s % 2
                        blk = 1 + s
                        is_last_even = s == 64
                        is_last_odd = s == 63
                        nc.tensor.matmul(
                            acc[64 * half : 64 * half + 64, :],
                            w_sb[:, blk * 64 : (blk + 1) * 64],
                            rhs,
                            start=(half == 1 and first_odd),
                            stop=(is_last_even or is_last_odd),
                            tile_position=(0, 64 * half),
                        )
                        if half == 1:
                            first_odd = False

                # ACT evacuates both PSUM halves; accumulating DMA adds the
                # odd half into DRAM (keeps DVE free for products)
                o_t = outp.tile([OUTPUT_DIM, TILE_B], f32, tag="o", name="o_t")
                o2_t = outp.tile([OUTPUT_DIM, TILE_B], f32, tag="o2", name="o2_t")
                nc.scalar.activation(
                    o_t[:],
                    acc[0:64, :],
                    mybir.ActivationFunctionType.Identity,
                    bias=b_sb[:, 0:1],
                )
                nc.scalar.copy(o2_t[:], acc[64:128, :])
                bs = slice(t * TILE_B, (t + 1) * TILE_B)
                nc.sync.dma_start(out_ext[:, bs], o_t[:])
                nc.gpsimd.dma_start(
                    out_ext[:, bs], o2_t[:], accum_op=mybir.AluOpType.add
                )

    _split_multiwaits(nc, mybir)

    # ---- per-core input maps ----
    in_maps = []
    for c in range(N_CORES):
        cs = xall[:, :, c * B_CORE : (c + 1) * B_CORE]  # [128, 16, 4096]
        xtiles = np.ascontiguousarray(
            cs.reshape(128, N_ROT, N_TILES, TILE_B).transpose(2, 0, 1, 3)
        )  # [N_TILES, 128, 16, TILE_B]
        in_maps.append(
            {
                "xall": xtiles,
                "Wd": w_packed,
                "bias": bias.reshape(OUTPUT_DIM, 1),
            }
        )
    return nc, in_maps


def kernel(x, W, b, indices_0, indices_1):
    from concourse.bass_utils import run_bass_kernel_spmd

    nc, in_maps = build(x, W, b)
    res = run_bass_kernel_spmd(nc, in_maps, list(range(N_CORES))).results
    out = np.concatenate([np.asarray(r["outT"], np.float32).T for r in res], axis=0)
    return out



# revision 3
# speedup vs baseline: 1.4981x; 1.4981x over previous
"""Polynomial features (degree 2) + linear layer, distributed over 8 TRN2 cores.

reference: A = [x, {x_i*x_j for i<=j}] (8384 coeffs); out = A @ W.T + b.

Hybrid kernel: each core processes 8 batch tiles of 512; each tile runs one
of two algorithms (TILE_MODE string, 'c'/'p'):

'c' (class) tiles - circular-distance-class products (DVE-bound):
  - pairs enumerated by distance class s in 0..64: class s, lane p ->
    {p, (p+s)%128}; host ships 16 rotated copies of x^T (fp16)
  - 65 class products via 8 grouped DVE tensor_mul ops; 66 K=128 matmuls
    (2-way column-tiled) accumulate into PSUM [64+64, 512]
'p' (pencil) tiles - congruence-pencil squared projections (ACT-heavy):
  - outputs paired; for each pair (S_a, S_b) of quadratic forms, a real
    congruence basis B gives S_a = B^T D_a B, S_b = B^T D_b B (2x2 blocks
    from complex pencil eigenvalues handled with one extra (y1+y2)
    projection) => out = sum_k g_k (v_k . x)^2, <=192 projections/pencil
  - 48 projection matmuls [128x128] -> PSUM; ACT Square evacuates to fp16
    z2 in SBUF; contraction: 1 linear matmul + 2 matmuls/pencil (K=128 +
    K=64) into a 4-way column-tiled PSUM accumulator
  - fp16 throughout (bf16 fails: the pencil basis amplifies quantization
    ~40x; fp16 measured max rel err ~1.1e-2 vs gate 2e-2)

This splits the elementwise work (the bottleneck) between DVE (class
products) and ACT (pencil squares); GpSimd is avoided for compute (SBUF
port contention with DVE measured a 1.55x slowdown).

TPB instructions have a single sync-wait slot, but Tile emits multiple
waits on slot-recycling instructions; _split_multiwaits() post-processes
the BIR, hoisting extra waits onto injected same-engine NOPs.
"""

import os

import numpy as np
import ml_dtypes

INPUT_DIM = 128
OUTPUT_DIM = 64
BATCH = 32768
N_CORES = 8
B_CORE = BATCH // N_CORES  # 4096
TILE_B = 512
N_TILES = B_CORE // TILE_B  # 8

TILE_MODE = os.environ.get("K_TILE_MODE", "cpccpccp")
assert len(TILE_MODE) == N_TILES and set(TILE_MODE) <= {"c", "p"}
N_C = TILE_MODE.count("c")
N_P = TILE_MODE.count("p")

# every k-th 2-block square goes to DVE instead of ACT (0 = all ACT)
SQ_DVE = int(os.environ.get("K_SQ_DVE", "0"))

ROT_SET = [0, 1, 2, 3, 4, 5, 6, 7, 8, 16, 24, 32, 40, 48, 56, 64]
N_ROT = len(ROT_SET)
ROT_IDX = {d: i for i, d in enumerate(ROT_SET)}

N_PENCIL = OUTPUT_DIM // 2  # 32
R_PAD = 192                 # max 128 + 64 (all-complex pencil) exactly fits
N_PROJ = N_PENCIL * R_PAD   # 6144
N_PBLK = N_PROJ // 128      # 48 projection matmul blocks


def _class_ops():
    """(a, b) rotation pair per distance class s=0..64 with b - a = s."""
    ops = []
    for s in range(65):
        if s <= 8:
            a, b = 0, s
        else:
            k = (s - 1) // 8  # 1..7
            anchor = 8 * k + 8
            a, b = anchor - s, anchor
        assert a in ROT_SET and b in ROT_SET and b - a == s, (s, a, b)
        ops.append((a, b))
    return ops


CLASS_OPS = _class_ops()


def _row_of_output(o):
    """PSUM partition row of output o in the pencil contraction layout."""
    p, h = o // 2, o % 2
    return 32 * (p % 4) + 2 * (p // 4) + h


# ---------------------------------------------------------------------------
# host: class-path weight packing (identical to the class-only kernel)
# ---------------------------------------------------------------------------

def _build_device_weights(W, b):
    """Permute W [64, 8384] into the class K-block layout [128, 66*64]."""
    W = np.asarray(W, np.float32)
    n = INPUT_DIM
    pair_off = {}
    c = 0
    for i in range(n):
        for j in range(i, n):
            pair_off[(i, j)] = c
            c += 1
    assert c == 8256

    Wd = np.zeros((66, 128, OUTPUT_DIM), np.float32)
    Wd[0] = W[:, 0:128].T  # linear block
    seen = set()
    for s in range(65):
        a, _bb = CLASS_OPS[s]
        for p in range(128):
            u = (p + a) % 128
            v = (p + a + s) % 128
            i, j = (u, v) if u <= v else (v, u)
            if (i, j) in seen:
                continue  # duplicate lane (s=64 second half)
            seen.add((i, j))
            Wd[1 + s, p] = W[:, 128 + pair_off[(i, j)]]
    assert len(seen) == 8256, len(seen)
    w_packed = np.ascontiguousarray(
        Wd.transpose(1, 0, 2).reshape(128, 66 * OUTPUT_DIM)
    ).astype(np.float16)
    return w_packed, np.asarray(b, np.float32)


# ---------------------------------------------------------------------------
# host: pencil decomposition
# ---------------------------------------------------------------------------

def _build_S(W2):
    """W2 [64, 8256] -> S [64,128,128] symmetric with x^T S_o x = sum W2 x_i x_j."""
    n = INPUT_DIM
    iu = np.triu_indices(n)
    S = np.zeros((OUTPUT_DIM, n, n))
    for o in range(OUTPUT_DIM):
        M = np.zeros((n, n))
        M[iu] = W2[o]
        S[o] = (M + M.T) / 2
    return S


def _pencil_decompose(Sa, Sb):
    """V [R,128] (unit rows), ga, gb [R]: x^T Sa x = sum ga_k (V_k.x)^2 etc."""
    n = Sa.shape[0]
    M = np.linalg.solve(Sb, Sa)
    lam, Vc = np.linalg.eig(M)
    cols = []
    used = np.zeros(n, bool)
    for i in range(n):
        if used[i]:
            continue
        if abs(lam[i].imag) < 1e-9 * max(1.0, abs(lam[i].real)):
            cols.append(Vc[:, i].real)
            used[i] = True
        else:
            rest = [k for k in range(i + 1, n) if not used[k]]
            j = min(rest, key=lambda k: abs(lam[k] - lam[i].conjugate()))
            cols.append(Vc[:, i].real)
            cols.append(Vc[:, i].imag)
            used[i] = used[j] = True
    X = np.stack(cols, axis=1)
    A = X.T @ Sa @ X
    Bm = X.T @ Sb @ X
    Vrows = np.linalg.inv(X)
    proj, ga, gb = [], [], []
    scale_a = np.abs(A).max()
    k = 0
    while k < n:
        if k + 1 < n and (abs(A[k, k + 1]) > 1e-8 * scale_a
                          or abs(Bm[k, k + 1]) > 1e-8 * scale_a):
            A2 = A[k:k + 2, k:k + 2]
            B2 = Bm[k:k + 2, k:k + 2]
            w, R = np.linalg.eigh(A2)  # rotate to diagonalize the A block
            B2r = R.T @ B2 @ R
            r1 = R[0, 0] * Vrows[k] + R[1, 0] * Vrows[k + 1]
            r2 = R[0, 1] * Vrows[k] + R[1, 1] * Vrows[k + 1]
            b12 = B2r[0, 1]
            # 2 b12 y1 y2 = b12[(y1+y2)^2 - y1^2 - y2^2]
            proj += [r1, r2, r1 + r2]
            ga += [w[0], w[1], 0.0]
            gb += [B2r[0, 0] - b12, B2r[1, 1] - b12, b12]
            k += 2
        else:
            proj.append(Vrows[k])
            ga.append(A[k, k])
            gb.append(Bm[k, k])
            k += 1
    V = np.stack(proj, axis=0)
    ga = np.asarray(ga)
    gb = np.asarray(gb)
    nrm = np.linalg.norm(V, axis=1)
    V = V / nrm[:, None]
    return V, ga * nrm**2, gb * nrm**2


def _build_pencil_weights(W, b):
    """Pack pencil projection/contraction tensors.

    Returns Vd [128, N_PROJ] fp16 (lhsT: feature x flat-proj),
    Gd [128, 64*32] fp16 (contraction stationaries, 32-wide strips),
    W1L [128, 128] fp16 (linear term -> permuted out rows),
    bias_p [128, 1] f32.
    """
    W = np.asarray(W, np.float64)
    b = np.asarray(b, np.float64)
    W1, W2 = W[:, :128], W[:, 128:]
    S = _build_S(W2)

    Vflat = np.zeros((N_PROJ, 128))
    Gaf = np.zeros(N_PROJ)
    Gbf = np.zeros(N_PROJ)
    for p in range(N_PENCIL):
        V, ga, gb = _pencil_decompose(S[2 * p], S[2 * p + 1])
        R = V.shape[0]
        assert R <= R_PAD, R
        Vflat[R_PAD * p:R_PAD * p + R] = V
        Gaf[R_PAD * p:R_PAD * p + R] = ga
        Gbf[R_PAD * p:R_PAD * p + R] = gb

    Vd = np.ascontiguousarray(Vflat.T).astype(np.float16)

    Gd = np.zeros((128, 2 * N_PENCIL * 32), np.float64)
    for p in range(N_PENCIL):
        s, i = p % 4, p // 4
        ca, cb = 2 * i, 2 * i + 1
        loc = slice(R_PAD * p, R_PAD * (p + 1))
        ga, gb = Gaf[loc], Gbf[loc]
        m0, m1 = 2 * p, 2 * p + 1
        if p % 2 == 0:
            # rows 0:128 -> block 3p/2 full; rows 128:192 -> next block [0:64)
            Gd[0:128, m0 * 32 + ca] = ga[0:128]
            Gd[0:128, m0 * 32 + cb] = gb[0:128]
            Gd[0:64, m1 * 32 + ca] = ga[128:192]
            Gd[0:64, m1 * 32 + cb] = gb[128:192]
        else:
            # rows 0:64 -> block (3p-1)/2 [64:128); rows 64:192 -> next full
            Gd[64:128, m0 * 32 + ca] = ga[0:64]
            Gd[64:128, m0 * 32 + cb] = gb[0:64]
            Gd[0:128, m1 * 32 + ca] = ga[64:192]
            Gd[0:128, m1 * 32 + cb] = gb[64:192]
    Gd = Gd.astype(np.float16)

    W1L = np.zeros((128, 128), np.float64)
    bias_p = np.zeros((128, 1), np.float64)
    for o in range(OUTPUT_DIM):
        r = _row_of_output(o)
        W1L[:, r] = W1[o]
        bias_p[r, 0] = b[o]
    return Vd, Gd, W1L.astype(np.float16), bias_p.astype(np.float32)


def _split_multiwaits(nc, mybir):
    """TPB instructions have one sync-wait slot; hoist extras onto NOPs."""
    import bass_rust

    n_split = 0
    for fn in nc.m.functions:
        for bb in fn.blocks:
            out = []
            changed = False
            for inst in bb.instructions:
                si = getattr(inst, "sync_info", None)
                if si is not None and si.on_wait and len(si.on_wait) > 1:
                    for w in si.on_wait[:-1]:
                        n_split += 1
                        nop = bass_rust.InstNoOp(
                            name=f"I-mw{n_split}",
                            engine=inst.engine,
                            ins=[],
                            outs=[],
                            sync_info=mybir.SyncInfo(on_wait=[w], on_update=[]),
                            bass_nofuse=True,
                        )
                        out.append(nop)
                    inst.sync_info = mybir.SyncInfo(
                        on_wait=[si.on_wait[-1]], on_update=si.on_update
                    )
                    changed = True
                out.append(inst)
            if changed:
                bb.instructions = out
    return n_split


def build(x, W, b):
    """Build the Bass graph and per-core input maps. Returns (nc, in_maps)."""
    import concourse.bass as bass
    import concourse.mybir as mybir
    from concourse import tile

    f16 = mybir.dt.float16
    f32 = mybir.dt.float32

    # ---- host preprocessing ----
    xT = np.ascontiguousarray(np.asarray(x, np.float32).T).astype(np.float16)
    # xall[p, i, n] = feature (p + ROT_SET[i]) % 128 of sample n
    xall = np.stack([np.roll(xT, -d, axis=0) for d in ROT_SET], axis=1)
    w_packed, bias = _build_device_weights(W, b)
    Vd, Gd, W1L, bias_p = _build_pencil_weights(W, b)

    # ---- device graph ----
    nc = bass.Bass()
    if N_C:
        xc_in = nc.declare_dram_parameter(
            "xallc", [N_C, 128, N_ROT, TILE_B], f16, isOutput=False
        )
        outc_ext = nc.declare_dram_parameter(
            "outc", [OUTPUT_DIM, N_C * TILE_B], f32, isOutput=True
        )
    if N_P:
        xp_in = nc.declare_dram_parameter(
            "xpen", [N_P, 128, TILE_B], f16, isOutput=False
        )
        v_in = nc.declare_dram_parameter("Vd", [128, N_PROJ], f16, isOutput=False)
        g_in = nc.declare_dram_parameter(
            "Gd", [128, 2 * N_PENCIL * 32], f16, isOutput=False
        )
        w1l_in = nc.declare_dram_parameter("W1L", [128, 128], f16, isOutput=False)
        pb_in = nc.declare_dram_parameter("biasp", [128, 1], f32, isOutput=False)
        outp_ext = nc.declare_dram_parameter(
            "outp", [128, N_P * TILE_B], f32, isOutput=True
        )
    w_in = nc.declare_dram_parameter("Wd", [128, 66 * 64], f16, isOutput=False)
    b_in = nc.declare_dram_parameter("bias", [OUTPUT_DIM, 1], f32, isOutput=False)

    # multi-class ops: one per anchor family, constant-stride rotation APs
    MC_OPS = [list(range(0, 9))] + [
        list(range(8 * k + 1, 8 * k + 9)) for k in range(1, 8)
    ]

    def rot_group_ap(xrt, classes):
        """[128, len(classes), TILE_B] APs (in0, in1)."""
        m = len(classes)
        us = [ROT_IDX[CLASS_OPS[s][0]] for s in classes]
        vs = [ROT_IDX[CLASS_OPS[s][1]] for s in classes]

        def mk(idx):
            if all(i == idx[0] for i in idx):
                return xrt[:, idx[0]: idx[0] + 1, :].to_broadcast(
                    [128, m, TILE_B]
                )
            d = idx[1] - idx[0]
            assert all(idx[j + 1] - idx[j] == d for j in range(m - 1)), idx
            return xrt[:, idx[0]:: d, :][:, 0:m, :]

        return mk(us), mk(vs)

    with tile.TileContext(nc) as tc:
        with (
            tc.tile_pool(name="consts", bufs=1) as consts,
            tc.tile_pool(name="xc", bufs=2) as xcp,
            tc.tile_pool(name="xp", bufs=2) as xpp,
            tc.tile_pool(name="prod", bufs=3) as prodp,
            tc.tile_pool(name="z2", bufs=(2 if N_C == 0 else 1)) as z2p,
            tc.tile_pool(name="outp", bufs=3) as outp,
            tc.tile_pool(name="proj", bufs=3, space="PSUM") as projp,
            tc.tile_pool(name="acc", bufs=2, space="PSUM") as accp,
        ):
            w_sb = consts.tile([128, 66 * 64], f16)
            nc.sync.dma_start(w_sb[:], w_in[:])
            b_sb = consts.tile([OUTPUT_DIM, 1], f32)
            nc.sync.dma_start(b_sb[:], b_in[:])
            if N_P:
                v_sb = consts.tile([128, N_PROJ], f16)
                nc.sync.dma_start(v_sb[:], v_in[:])
                g_sb = consts.tile([128, 2 * N_PENCIL * 32], f16)
                nc.sync.dma_start(g_sb[:], g_in[:])
                w1l_sb = consts.tile([128, 128], f16)
                nc.sync.dma_start(w1l_sb[:], w1l_in[:])
                pb_sb = consts.tile([128, 1], f32)
                nc.sync.dma_start(pb_sb[:], pb_in[:])

            in_tiles = [None] * (N_TILES + 2)

            def load_input(t):
                if t >= N_TILES:
                    return
                mode = TILE_MODE[t]
                idx = TILE_MODE[:t].count(mode)
                if mode == "c":
                    xt = xcp.tile([128, N_ROT, TILE_B], f16, tag="xc", name="xc_t")
                    nc.sync.dma_start(xt[:], xc_in[idx][:])
                else:
                    xt = xpp.tile([128, TILE_B], f16, tag="xp", name="xp_t")
                    nc.sync.dma_start(xt[:], xp_in[idx][:])
                in_tiles[t] = xt

            def class_tile(xrt, ic):
                # acc halves: even classes + linear -> partitions 0:64,
                # odd classes -> partitions 64:128
                acc = accp.tile([128, TILE_B], f32, name="acc")
                nc.tensor.matmul(
                    acc[0:64, :],
                    w_sb[:, 0:64],
                    xrt[:, 0, :],
                    start=True,
                    stop=False,
                    tile_position=(0, 0),
                )
                first_odd = True
                for k, classes in enumerate(MC_OPS):
                    m = len(classes)
                    p_t = prodp.tile(
                        [128, m, TILE_B], f16, tag="prod" + str(m), name="p_t"
                    )
                    in0, in1 = rot_group_ap(xrt, classes)
                    nc.vector.tensor_mul(p_t[:], in0, in1)
                    for j, s in enumerate(classes):
                        half = s % 2
                        blk = 1 + s
                        nc.tensor.matmul(
                            acc[64 * half: 64 * half + 64, :],
                            w_sb[:, blk * 64: (blk + 1) * 64],
                            p_t[:, j, :],
                            start=(half == 1 and first_odd),
                            stop=(s == 64 or s == 63),
                            tile_position=(0, 64 * half),
                        )
                        if half == 1:
                            first_odd = False

                # ACT evacuates both PSUM halves; accumulating DMA adds the
                # odd half into DRAM (keeps DVE free for products)
                o_t = outp.tile([OUTPUT_DIM, TILE_B], f32, tag="o", name="o_t")
                o2_t = outp.tile([OUTPUT_DIM, TILE_B], f32, tag="o2", name="o2_t")
                nc.scalar.activation(
                    o_t[:],
                    acc[0:64, :],
                    mybir.ActivationFunctionType.Identity,
                    bias=b_sb[:, 0:1],
                )
                nc.scalar.copy(o2_t[:], acc[64:128, :])
                bs = slice(ic * TILE_B, (ic + 1) * TILE_B)
                nc.sync.dma_start(outc_ext[:, bs], o_t[:])
                nc.gpsimd.dma_start(
                    outc_ext[:, bs], o2_t[:], accum_op=mybir.AluOpType.add
                )

            def pencil_tile(x_t, ip):
                # 48 projection matmuls -> PSUM pairs -> Square -> z2 (fp16)
                z2 = z2p.tile([128, N_PBLK, TILE_B], f16, tag="z2", name="z2_t")
                for jj in range(N_PBLK // 2):
                    pj = projp.tile([128, 2, TILE_B], f32, name="pj")
                    for h in range(2):
                        j = 2 * jj + h
                        nc.tensor.matmul(
                            pj[:, h, :],
                            v_sb[:, j * 128: (j + 1) * 128],
                            x_t[:],
                            start=True,
                            stop=True,
                        )
                    dst = z2[:, 2 * jj: 2 * jj + 2, :]
                    if SQ_DVE and jj % SQ_DVE == SQ_DVE - 1:
                        nc.vector.tensor_mul(dst, pj[:], pj[:])
                    else:
                        nc.scalar.activation(
                            dst, pj[:], mybir.ActivationFunctionType.Square
                        )

                # contraction: linear (full width) + 2 matmuls per pencil
                # into 4-way column-tiled accumulator
                acc = accp.tile([128, TILE_B], f32, name="acc")
                nc.tensor.matmul(
                    acc[:], w1l_sb[:], x_t[:], start=True, stop=False,
                    tile_position=(0, 0),
                )
                for i in range(8):
                    for s in range(4):
                        p = 4 * i + s
                        last = i == 7
                        m0, m1 = 2 * p, 2 * p + 1
                        blkA = (3 * p) // 2
                        out_ap = acc[32 * s: 32 * s + 32, :]
                        if p % 2 == 0:
                            nc.tensor.matmul(
                                out_ap,
                                g_sb[0:128, m0 * 32: m0 * 32 + 32],
                                z2[:, blkA, :],
                                start=False, stop=False,
                                tile_position=(0, 32 * s),
                            )
                            nc.tensor.matmul(
                                out_ap,
                                g_sb[0:64, m1 * 32: m1 * 32 + 32],
                                z2[0:64, blkA + 1, :],
                                start=False, stop=last,
                                tile_position=(0, 32 * s),
                            )
                        else:
                            nc.tensor.matmul(
                                out_ap,
                                g_sb[64:128, m0 * 32: m0 * 32 + 32],
                                z2[64:128, blkA, :],
                                start=False, stop=False,
                                tile_position=(64, 32 * s),
                            )
                            nc.tensor.matmul(
                                out_ap,
                                g_sb[0:128, m1 * 32: m1 * 32 + 32],
                                z2[:, blkA + 1, :],
                                start=False, stop=last,
                                tile_position=(0, 32 * s),
                            )

                o_t = outp.tile([128, TILE_B], f32, tag="op", name="op_t")
                nc.scalar.activation(
                    o_t[:],
                    acc[:],
                    mybir.ActivationFunctionType.Identity,
                    bias=pb_sb[:, 0:1],
                )
                bs = slice(ip * TILE_B, (ip + 1) * TILE_B)
                nc.sync.dma_start(outp_ext[:, bs], o_t[:])

            load_input(0)
            load_input(1)
            for t in range(N_TILES):
                load_input(t + 2)
                mode = TILE_MODE[t]
                idx = TILE_MODE[:t].count(mode)
                if mode == "c":
                    class_tile(in_tiles[t], idx)
                else:
                    pencil_tile(in_tiles[t], idx)

    _split_multiwaits(nc, mybir)

    # ---- per-core input maps ----
    c_tiles = [t for t in range(N_TILES) if TILE_MODE[t] == "c"]
    p_tiles = [t for t in range(N_TILES) if TILE_MODE[t] == "p"]
    in_maps = []
    for c in range(N_CORES):
        base = c * B_CORE
        m = {"Wd": w_packed, "bias": bias.reshape(OUTPUT_DIM, 1)}
        if N_C:
            xc = np.stack([
                xall[:, :, base + t * TILE_B: base + (t + 1) * TILE_B]
                for t in c_tiles
            ])  # [N_C, 128, 16, TILE_B]
            m["xallc"] = np.ascontiguousarray(xc)
        if N_P:
            xp = np.stack([
                xT[:, base + t * TILE_B: base + (t + 1) * TILE_B]
                for t in p_tiles
            ])  # [N_P, 128, TILE_B]
            m["xpen"] = np.ascontiguousarray(xp)
            m["Vd"] = Vd
            m["Gd"] = Gd
            m["W1L"] = W1L
            m["biasp"] = bias_p
        in_maps.append(m)
    return nc, in_maps


def kernel(x, W, b, indices_0, indices_1):
    from concourse.bass_utils import run_bass_kernel_spmd

    nc, in_maps = build(x, W, b)
    res = run_bass_kernel_spmd(nc, in_maps, list(range(N_CORES))).results

    row_of_o = np.array([_row_of_output(o) for o in range(OUTPUT_DIM)])
    c_tiles = [t for t in range(N_TILES) if TILE_MODE[t] == "c"]
    p_tiles = [t for t in range(N_TILES) if TILE_MODE[t] == "p"]

    out = np.empty((BATCH, OUTPUT_DIM), np.float32)
    for c in range(N_CORES):
        base = c * B_CORE
        if N_C:
            outc = np.asarray(res[c]["outc"], np.float32)  # [64, N_C*512]
            for ic, t in enumerate(c_tiles):
                blk = outc[:, ic * TILE_B: (ic + 1) * TILE_B]
                out[base + t * TILE_B: base + (t + 1) * TILE_B] = blk.T
        if N_P:
            outp = np.asarray(res[c]["outp"], np.float32)  # [128, N_P*512]
            for ip, t in enumerate(p_tiles):
                blk = outp[row_of_o, ip * TILE_B: (ip + 1) * TILE_B]
                out[base + t * TILE_B: base + (t + 1) * TILE_B] = blk.T
    return out


# revision 8
# speedup vs baseline: 1.5536x; 1.0371x over previous
"""Polynomial features (degree 2) + linear layer, distributed over 8 TRN2 cores.

reference: A = [x, {x_i*x_j for i<=j}] (8384 coeffs); out = A @ W.T + b.

Hybrid kernel: each core processes 8 batch tiles of 512; each tile runs one
of two algorithms (TILE_MODE string, 'c'/'p'):

'c' (class) tiles - circular-distance-class products (DVE-bound):
  - pairs enumerated by distance class s in 0..64: class s, lane p ->
    {p, (p+s)%128}; host ships 16 rotated copies of x^T (fp16)
  - 65 class products via 8 grouped DVE tensor_mul ops; 66 K=128 matmuls
    (2-way column-tiled) accumulate into PSUM [64+64, 512]
'p' (pencil) tiles - congruence-pencil squared projections (ACT-heavy):
  - outputs paired; for each pair (S_a, S_b) of quadratic forms, a real
    congruence basis B gives S_a = B^T D_a B, S_b = B^T D_b B (2x2 blocks
    from complex pencil eigenvalues handled with one extra (y1+y2)
    projection) => out = sum_k g_k (v_k . x)^2, <=192 projections/pencil
  - 48 projection matmuls [128x128] -> PSUM; ACT Square evacuates to fp16
    z2 in SBUF; contraction: 1 linear matmul + 2 matmuls/pencil (K=128 +
    K=64) into a 4-way column-tiled PSUM accumulator
  - fp16 throughout (bf16 fails: the pencil basis amplifies quantization
    ~40x; fp16 measured max rel err ~1.1e-2 vs gate 2e-2)

This splits the elementwise work (the bottleneck) between DVE (class
products) and ACT (pencil squares); GpSimd is avoided for compute (SBUF
port contention with DVE measured a 1.55x slowdown).

TPB instructions have a single sync-wait slot, but Tile emits multiple
waits on slot-recycling instructions; _split_multiwaits() post-processes
the BIR, hoisting extra waits onto injected same-engine NOPs.
"""

import os

import numpy as np
import ml_dtypes

INPUT_DIM = 128
OUTPUT_DIM = 64
BATCH = 32768
N_CORES = 8
B_CORE = BATCH // N_CORES  # 4096
TILE_B = 512
N_TILES = B_CORE // TILE_B  # 8

TILE_MODE = os.environ.get("K_TILE_MODE", "cpcpcpcc")
assert len(TILE_MODE) == N_TILES and set(TILE_MODE) <= {"c", "p"}
N_C = TILE_MODE.count("c")
N_P = TILE_MODE.count("p")

# every k-th 2-block square goes to DVE instead of ACT (0 = all ACT)
SQ_DVE = int(os.environ.get("K_SQ_DVE", "0"))

ROT_SET = [0, 1, 2, 3, 4, 5, 6, 7, 8, 16, 24, 32, 40, 48, 56, 64]
N_ROT = len(ROT_SET)
ROT_IDX = {d: i for i, d in enumerate(ROT_SET)}

N_PENCIL = OUTPUT_DIM // 2  # 32
R_PAD = 192                 # max 128 + 64 (all-complex pencil) exactly fits
N_PROJ = N_PENCIL * R_PAD   # 6144
N_PBLK = N_PROJ // 128      # 48 projection matmul blocks


def _class_ops():
    """(a, b) rotation pair per distance class s=0..64 with b - a = s."""
    ops = []
    for s in range(65):
        if s <= 8:
            a, b = 0, s
        else:
            k = (s - 1) // 8  # 1..7
            anchor = 8 * k + 8
            a, b = anchor - s, anchor
        assert a in ROT_SET and b in ROT_SET and b - a == s, (s, a, b)
        ops.append((a, b))
    return ops


CLASS_OPS = _class_ops()


def _row_of_output(o):
    """PSUM partition row of output o in the pencil contraction layout."""
    p, h = o // 2, o % 2
    return 32 * (p % 4) + 2 * (p // 4) + h


# ---------------------------------------------------------------------------
# host: class-path weight packing (identical to the class-only kernel)
# ---------------------------------------------------------------------------

def _build_device_weights(W, b):
    """Permute W [64, 8384] into the class K-block layout [128, 66*64]."""
    W = np.asarray(W, np.float32)
    n = INPUT_DIM
    pair_off = {}
    c = 0
    for i in range(n):
        for j in range(i, n):
            pair_off[(i, j)] = c
            c += 1
    assert c == 8256

    Wd = np.zeros((66, 128, OUTPUT_DIM), np.float32)
    Wd[0] = W[:, 0:128].T  # linear block
    seen = set()
    for s in range(65):
        a, _bb = CLASS_OPS[s]
        for p in range(128):
            u = (p + a) % 128
            v = (p + a + s) % 128
            i, j = (u, v) if u <= v else (v, u)
            if (i, j) in seen:
                continue  # duplicate lane (s=64 second half)
            seen.add((i, j))
            Wd[1 + s, p] = W[:, 128 + pair_off[(i, j)]]
    assert len(seen) == 8256, len(seen)
    w_packed = np.ascontiguousarray(
        Wd.transpose(1, 0, 2).reshape(128, 66 * OUTPUT_DIM)
    ).astype(np.float16)
    return w_packed, np.asarray(b, np.float32)


# ---------------------------------------------------------------------------
# host: pencil decomposition
# ---------------------------------------------------------------------------

def _build_S(W2):
    """W2 [64, 8256] -> S [64,128,128] symmetric with x^T S_o x = sum W2 x_i x_j."""
    n = INPUT_DIM
    iu = np.triu_indices(n)
    S = np.zeros((OUTPUT_DIM, n, n))
    for o in range(OUTPUT_DIM):
        M = np.zeros((n, n))
        M[iu] = W2[o]
        S[o] = (M + M.T) / 2
    return S


def _pencil_decompose(Sa, Sb):
    """V [R,128] (unit rows), ga, gb [R]: x^T Sa x = sum ga_k (V_k.x)^2 etc."""
    n = Sa.shape[0]
    M = np.linalg.solve(Sb, Sa)
    lam, Vc = np.linalg.eig(M)
    cols = []
    used = np.zeros(n, bool)
    for i in range(n):
        if used[i]:
            continue
        if abs(lam[i].imag) < 1e-9 * max(1.0, abs(lam[i].real)):
            cols.append(Vc[:, i].real)
            used[i] = True
        else:
            rest = [k for k in range(i + 1, n) if not used[k]]
            j = min(rest, key=lambda k: abs(lam[k] - lam[i].conjugate()))
            cols.append(Vc[:, i].real)
            cols.append(Vc[:, i].imag)
            used[i] = used[j] = True
    X = np.stack(cols, axis=1)
    A = X.T @ Sa @ X
    Bm = X.T @ Sb @ X
    Vrows = np.linalg.inv(X)
    proj, ga, gb = [], [], []
    scale_a = np.abs(A).max()
    k = 0
    while k < n:
        if k + 1 < n and (abs(A[k, k + 1]) > 1e-8 * scale_a
                          or abs(Bm[k, k + 1]) > 1e-8 * scale_a):
            A2 = A[k:k + 2, k:k + 2]
            B2 = Bm[k:k + 2, k:k + 2]
            w, R = np.linalg.eigh(A2)  # rotate to diagonalize the A block
            B2r = R.T @ B2 @ R
            r1 = R[0, 0] * Vrows[k] + R[1, 0] * Vrows[k + 1]
            r2 = R[0, 1] * Vrows[k] + R[1, 1] * Vrows[k + 1]
            b12 = B2r[0, 1]
            # 2 b12 y1 y2 = b12[(y1+y2)^2 - y1^2 - y2^2]
            proj += [r1, r2, r1 + r2]
            ga += [w[0], w[1], 0.0]
            gb += [B2r[0, 0] - b12, B2r[1, 1] - b12, b12]
            k += 2
        else:
            proj.append(Vrows[k])
            ga.append(A[k, k])
            gb.append(Bm[k, k])
            k += 1
    V = np.stack(proj, axis=0)
    ga = np.asarray(ga)
    gb = np.asarray(gb)
    nrm = np.linalg.norm(V, axis=1)
    V = V / nrm[:, None]
    return V, ga * nrm**2, gb * nrm**2


def _build_pencil_weights(W, b):
    """Pack pencil projection/contraction tensors.

    Returns Vd [128, N_PROJ] fp16 (lhsT: feature x flat-proj),
    Gd [128, 64*32] fp16 (contraction stationaries, 32-wide strips),
    W1L [128, 128] fp16 (linear term -> permuted out rows),
    bias_p [128, 1] f32.
    """
    W = np.asarray(W, np.float64)
    b = np.asarray(b, np.float64)
    W1, W2 = W[:, :128], W[:, 128:]
    S = _build_S(W2)

    Vflat = np.zeros((N_PROJ, 128))
    Gaf = np.zeros(N_PROJ)
    Gbf = np.zeros(N_PROJ)
    for p in range(N_PENCIL):
        V, ga, gb = _pencil_decompose(S[2 * p], S[2 * p + 1])
        R = V.shape[0]
        assert R <= R_PAD, R
        Vflat[R_PAD * p:R_PAD * p + R] = V
        Gaf[R_PAD * p:R_PAD * p + R] = ga
        Gbf[R_PAD * p:R_PAD * p + R] = gb

    Vd = np.ascontiguousarray(Vflat.T).astype(np.float16)

    Gd = np.zeros((128, 2 * N_PENCIL * 32), np.float64)
    for p in range(N_PENCIL):
        s, i = p % 4, p // 4
        ca, cb = 2 * i, 2 * i + 1
        loc = slice(R_PAD * p, R_PAD * (p + 1))
        ga, gb = Gaf[loc], Gbf[loc]
        m0, m1 = 2 * p, 2 * p + 1
        if p % 2 == 0:
            # rows 0:128 -> block 3p/2 full; rows 128:192 -> next block [0:64)
            Gd[0:128, m0 * 32 + ca] = ga[0:128]
            Gd[0:128, m0 * 32 + cb] = gb[0:128]
            Gd[0:64, m1 * 32 + ca] = ga[128:192]
            Gd[0:64, m1 * 32 + cb] = gb[128:192]
        else:
            # rows 0:64 -> block (3p-1)/2 [64:128); rows 64:192 -> next full
            Gd[64:128, m0 * 32 + ca] = ga[0:64]
            Gd[64:128, m0 * 32 + cb] = gb[0:64]
            Gd[0:128, m1 * 32 + ca] = ga[64:192]
            Gd[0:128, m1 * 32 + cb] = gb[64:192]
    Gd = Gd.astype(np.float16)

    W1L = np.zeros((128, 128), np.float64)
    bias_p = np.zeros((128, 1), np.float64)
    for o in range(OUTPUT_DIM):
        r = _row_of_output(o)
        W1L[:, r] = W1[o]
        bias_p[r, 0] = b[o]
    return Vd, Gd, W1L.astype(np.float16), bias_p.astype(np.float32)


def _split_multiwaits(nc, mybir):
    """TPB instructions have one sync-wait slot; hoist extras onto NOPs."""
    import bass_rust

    n_split = 0
    for fn in nc.m.functions:
        for bb in fn.blocks:
            out = []
            changed = False
            for inst in bb.instructions:
                si = getattr(inst, "sync_info", None)
                if si is not None and si.on_wait and len(si.on_wait) > 1:
                    for w in si.on_wait[:-1]:
                        n_split += 1
                        nop = bass_rust.InstNoOp(
                            name=f"I-mw{n_split}",
                            engine=inst.engine,
                            ins=[],
                            outs=[],
                            sync_info=mybir.SyncInfo(on_wait=[w], on_update=[]),
                            bass_nofuse=True,
                        )
                        out.append(nop)
                    inst.sync_info = mybir.SyncInfo(
                        on_wait=[si.on_wait[-1]], on_update=si.on_update
                    )
                    changed = True
                out.append(inst)
            if changed:
                bb.instructions = out
    return n_split


def build(x, W, b):
    """Build the Bass graph and per-core input maps. Returns (nc, in_maps)."""
    import concourse.bass as bass
    import concourse.mybir as mybir
    from concourse import tile

    f16 = mybir.dt.float16
    f32 = mybir.dt.float32

    # ---- host preprocessing ----
    xT = np.ascontiguousarray(np.asarray(x, np.float32).T).astype(np.float16)
    # xall[p, i, n] = feature (p + ROT_SET[i]) % 128 of sample n
    xall = np.stack([np.roll(xT, -d, axis=0) for d in ROT_SET], axis=1)
    w_packed, bias = _build_device_weights(W, b)
    Vd, Gd, W1L, bias_p = _build_pencil_weights(W, b)

    # ---- device graph ----
    nc = bass.Bass()
    if N_C:
        xc_in = nc.declare_dram_parameter(
            "xallc", [N_C, 128, N_ROT, TILE_B], f16, isOutput=False
        )
        outc_ext = nc.declare_dram_parameter(
            "outc", [OUTPUT_DIM, N_C * TILE_B], f32, isOutput=True
        )
    if N_P:
        xp_in = nc.declare_dram_parameter(
            "xpen", [N_P, 128, TILE_B], f16, isOutput=False
        )
        v_in = nc.declare_dram_parameter("Vd", [128, N_PROJ], f16, isOutput=False)
        g_in = nc.declare_dram_parameter(
            "Gd", [128, 2 * N_PENCIL * 32], f16, isOutput=False
        )
        w1l_in = nc.declare_dram_parameter("W1L", [128, 128], f16, isOutput=False)
        pb_in = nc.declare_dram_parameter("biasp", [128, 1], f32, isOutput=False)
        outp_ext = nc.declare_dram_parameter(
            "outp", [128, N_P * TILE_B], f32, isOutput=True
        )
    w_in = nc.declare_dram_parameter("Wd", [128, 66 * 64], f16, isOutput=False)
    b_in = nc.declare_dram_parameter("bias", [OUTPUT_DIM, 1], f32, isOutput=False)

    # multi-class ops: one per anchor family, constant-stride rotation APs
    MC_OPS = [list(range(0, 9))] + [
        list(range(8 * k + 1, 8 * k + 9)) for k in range(1, 8)
    ]

    def rot_group_ap(xrt, classes):
        """[128, len(classes), TILE_B] APs (in0, in1)."""
        m = len(classes)
        us = [ROT_IDX[CLASS_OPS[s][0]] for s in classes]
        vs = [ROT_IDX[CLASS_OPS[s][1]] for s in classes]

        def mk(idx):
            if all(i == idx[0] for i in idx):
                return xrt[:, idx[0]: idx[0] + 1, :].to_broadcast(
                    [128, m, TILE_B]
                )
            d = idx[1] - idx[0]
            assert all(idx[j + 1] - idx[j] == d for j in range(m - 1)), idx
            return xrt[:, idx[0]:: d, :][:, 0:m, :]

        return mk(us), mk(vs)

    with tile.TileContext(nc) as tc:
        with (
            tc.tile_pool(name="consts", bufs=1) as consts,
            tc.tile_pool(name="xc", bufs=3) as xcp,
            tc.tile_pool(name="xp", bufs=2) as xpp,
            tc.tile_pool(name="prod", bufs=3) as prodp,
            tc.tile_pool(name="z2", bufs=(2 if N_C == 0 else 1)) as z2p,
            tc.tile_pool(name="outp", bufs=3) as outp,
            tc.tile_pool(name="proj", bufs=3, space="PSUM") as projp,
            tc.tile_pool(name="acc", bufs=2, space="PSUM") as accp,
        ):
            # pencil consts go on the GpSimd (SWDGE) queue so the first
            # input tiles aren't stuck behind 1.5MB of projection weights
            if N_P:
                v_sb = consts.tile([128, N_PROJ], f16)
                nc.gpsimd.dma_start(v_sb[:], v_in[:])
                g_sb = consts.tile([128, 2 * N_PENCIL * 32], f16)
                nc.gpsimd.dma_start(g_sb[:], g_in[:])
                w1l_sb = consts.tile([128, 128], f16)
                nc.gpsimd.dma_start(w1l_sb[:], w1l_in[:])
                pb_sb = consts.tile([128, 1], f32)
                nc.gpsimd.dma_start(pb_sb[:], pb_in[:])
            w_sb = consts.tile([128, 66 * 64], f16)
            b_sb = consts.tile([OUTPUT_DIM, 1], f32)

            loaded = {}

            def load_input(idx, mode):
                if mode == "c":
                    xt = xcp.tile([128, N_ROT, TILE_B], f16, tag="xc", name="xc_t")
                    nc.sync.dma_start(xt[:], xc_in[idx][:])
                else:
                    xt = xpp.tile([128, TILE_B], f16, tag="xp", name="xp_t")
                    nc.sync.dma_start(xt[:], xp_in[idx][:])
                loaded[(mode, idx)] = xt

            def class_tile(xrt, ic):
                # acc halves: even classes + linear -> partitions 0:64,
                # odd classes -> partitions 64:128
                acc = accp.tile([128, TILE_B], f32, name="acc")
                nc.tensor.matmul(
                    acc[0:64, :],
                    w_sb[:, 0:64],
                    xrt[:, 0, :],
                    start=True,
                    stop=False,
                    tile_position=(0, 0),
                )
                first_odd = True
                for k, classes in enumerate(MC_OPS):
                    m = len(classes)
                    p_t = prodp.tile(
                        [128, m, TILE_B], f16, tag="prod" + str(m), name="p_t"
                    )
                    in0, in1 = rot_group_ap(xrt, classes)
                    nc.vector.tensor_mul(p_t[:], in0, in1)
                    for j, s in enumerate(classes):
                        half = s % 2
                        blk = 1 + s
                        nc.tensor.matmul(
                            acc[64 * half: 64 * half + 64, :],
                            w_sb[:, blk * 64: (blk + 1) * 64],
                            p_t[:, j, :],
                            start=(half == 1 and first_odd),
                            stop=(s == 64 or s == 63),
                            tile_position=(0, 64 * half),
                        )
                        if half == 1:
                            first_odd = False

                # ACT evacuates both PSUM halves; accumulating DMA adds the
                # odd half into DRAM (keeps DVE free for products)
                o_t = outp.tile([OUTPUT_DIM, TILE_B], f32, tag="o", name="o_t")
                o2_t = outp.tile([OUTPUT_DIM, TILE_B], f32, tag="o2", name="o2_t")
                nc.scalar.activation(
                    o_t[:],
                    acc[0:64, :],
                    mybir.ActivationFunctionType.Identity,
                    bias=b_sb[:, 0:1],
                )
                nc.scalar.copy(o2_t[:], acc[64:128, :])
                bs = slice(ic * TILE_B, (ic + 1) * TILE_B)
                nc.sync.dma_start(outc_ext[:, bs], o_t[:])
                nc.gpsimd.dma_start(
                    outc_ext[:, bs], o2_t[:], accum_op=mybir.AluOpType.add
                )

            def pencil_projections(x_t):
                # 48 projection matmuls -> PSUM pairs -> Square -> z2 (fp16)
                z2 = z2p.tile([128, N_PBLK, TILE_B], f16, tag="z2", name="z2_t")
                for jj in range(N_PBLK // 2):
                    pj = projp.tile([128, 2, TILE_B], f32, name="pj")
                    for h in range(2):
                        j = 2 * jj + h
                        nc.tensor.matmul(
                            pj[:, h, :],
                            v_sb[:, j * 128: (j + 1) * 128],
                            x_t[:],
                            start=True,
                            stop=True,
                        )
                    dst = z2[:, 2 * jj: 2 * jj + 2, :]
                    if SQ_DVE and jj % SQ_DVE == SQ_DVE - 1:
                        nc.vector.tensor_mul(dst, pj[:], pj[:])
                    else:
                        nc.scalar.activation(
                            dst, pj[:], mybir.ActivationFunctionType.Square
                        )
                return z2

            def pencil_contraction(z2, x_t, ip):
                # contraction: linear (full width) + 2 matmuls per pencil
                # into 4-way column-tiled accumulator
                acc = accp.tile([128, TILE_B], f32, name="acc")
                nc.tensor.matmul(
                    acc[:], w1l_sb[:], x_t[:], start=True, stop=False,
                    tile_position=(0, 0),
                )
                for i in range(8):
                    for s in range(4):
                        p = 4 * i + s
                        last = i == 7
                        m0, m1 = 2 * p, 2 * p + 1
                        blkA = (3 * p) // 2
                        out_ap = acc[32 * s: 32 * s + 32, :]
                        if p % 2 == 0:
                            nc.tensor.matmul(
                                out_ap,
                                g_sb[0:128, m0 * 32: m0 * 32 + 32],
                                z2[:, blkA, :],
                                start=False, stop=False,
                                tile_position=(0, 32 * s),
                            )
                            nc.tensor.matmul(
                                out_ap,
                                g_sb[0:64, m1 * 32: m1 * 32 + 32],
                                z2[0:64, blkA + 1, :],
                                start=False, stop=last,
                                tile_position=(0, 32 * s),
                            )
                        else:
                            nc.tensor.matmul(
                                out_ap,
                                g_sb[64:128, m0 * 32: m0 * 32 + 32],
                                z2[64:128, blkA, :],
                                start=False, stop=False,
                                tile_position=(64, 32 * s),
                            )
                            nc.tensor.matmul(
                                out_ap,
                                g_sb[0:128, m1 * 32: m1 * 32 + 32],
                                z2[:, blkA + 1, :],
                                start=False, stop=last,
                                tile_position=(0, 32 * s),
                            )

                o_t = outp.tile([128, TILE_B], f32, tag="op", name="op_t")
                nc.scalar.activation(
                    o_t[:],
                    acc[:],
                    mybir.ActivationFunctionType.Identity,
                    bias=pb_sb[:, 0:1],
                )
                bs = slice(ip * TILE_B, (ip + 1) * TILE_B)
                nc.sync.dma_start(outp_ext[:, bs], o_t[:])

            # emission schedule: each pencil tile paired with a class tile.
            # Per pair: [projections + squares][class products + matmuls]
            # [pencil contraction][evacuations] - DVE chews class products
            # while ACT chews pencil squares, tensor serves both.
            units = []       # ('pair', ip, ic) | ('c', ic) | ('p', ip)
            n_pair = min(N_P, N_C)
            for i in range(n_pair):
                units.append(("pair", i, i))
            units += [("c", i) for i in range(n_pair, N_C)]
            units += [("p", i) for i in range(n_pair, N_P)]

            load_seq = []    # (mode, idx) in consumption order
            for u in units:
                if u[0] == "pair":
                    load_seq.append(("p", u[1]))
                    load_seq.append(("c", u[2]))
                else:
                    load_seq.append((u[0], u[1]))

            # first two inputs, then the class weights, then the rest ahead
            nload = 0

            def prefetch(n):
                nonlocal nload
                while nload < min(n, len(load_seq)):
                    load_input(load_seq[nload][1], load_seq[nload][0])
                    nload += 1

            prefetch(2)
            nc.sync.dma_start(w_sb[:], w_in[:])
            nc.sync.dma_start(b_sb[:], b_in[:])
            prefetch(4)

            done = 0
            for u in units:
                if u[0] == "pair":
                    _, ip, ic = u
                    z2 = pencil_projections(loaded[("p", ip)])
                    class_tile(loaded[("c", ic)], ic)
                    pencil_contraction(z2, loaded[("p", ip)], ip)
                    done += 2
                elif u[0] == "c":
                    class_tile(loaded[("c", u[1])], u[1])
                    done += 1
                else:
                    z2 = pencil_projections(loaded[("p", u[1])])
                    pencil_contraction(z2, loaded[("p", u[1])], u[1])
                    done += 1
                prefetch(done + 3)

    _split_multiwaits(nc, mybir)

    # ---- per-core input maps ----
    c_tiles = [t for t in range(N_TILES) if TILE_MODE[t] == "c"]
    p_tiles = [t for t in range(N_TILES) if TILE_MODE[t] == "p"]
    in_maps = []
    for c in range(N_CORES):
        base = c * B_CORE
        m = {"Wd": w_packed, "bias": bias.reshape(OUTPUT_DIM, 1)}
        if N_C:
            xc = np.stack([
                xall[:, :, base + t * TILE_B: base + (t + 1) * TILE_B]
                for t in c_tiles
            ])  # [N_C, 128, 16, TILE_B]
            m["xallc"] = np.ascontiguousarray(xc)
        if N_P:
            xp = np.stack([
                xT[:, base + t * TILE_B: base + (t + 1) * TILE_B]
                for t in p_tiles
            ])  # [N_P, 128, TILE_B]
            m["xpen"] = np.ascontiguousarray(xp)
            m["Vd"] = Vd
            m["Gd"] = Gd
            m["W1L"] = W1L
            m["biasp"] = bias_p
        in_maps.append(m)
    return nc, in_maps


def kernel(x, W, b, indices_0, indices_1):
    from concourse.bass_utils import run_bass_kernel_spmd

    nc, in_maps = build(x, W, b)
    res = run_bass_kernel_spmd(nc, in_maps, list(range(N_CORES))).results

    row_of_o = np.array([_row_of_output(o) for o in range(OUTPUT_DIM)])
    c_tiles = [t for t in range(N_TILES) if TILE_MODE[t] == "c"]
    p_tiles = [t for t in range(N_TILES) if TILE_MODE[t] == "p"]

    out = np.empty((BATCH, OUTPUT_DIM), np.float32)
    for c in range(N_CORES):
        base = c * B_CORE
        if N_C:
            outc = np.asarray(res[c]["outc"], np.float32)  # [64, N_C*512]
            for ic, t in enumerate(c_tiles):
                blk = outc[:, ic * TILE_B: (ic + 1) * TILE_B]
                out[base + t * TILE_B: base + (t + 1) * TILE_B] = blk.T
        if N_P:
            outp = np.asarray(res[c]["outp"], np.float32)  # [128, N_P*512]
            for ip, t in enumerate(p_tiles):
                blk = outp[row_of_o, ip * TILE_B: (ip + 1) * TILE_B]
                out[base + t * TILE_B: base + (t + 1) * TILE_B] = blk.T
    return out


# revision 17
# speedup vs baseline: 1.7930x; 1.1540x over previous
"""Polynomial features (degree 2) + linear layer, distributed over 8 TRN2 cores.

reference: A = [x, {x_i*x_j for i<=j}] (8384 coeffs); out = A @ W.T + b.

Hybrid kernel: each core processes 8 batch tiles of 512; each tile runs one
of two algorithms (TILE_MODE string, 'c'/'p'):

'c' (class) tiles - circular-distance-class products (DVE-bound):
  - pairs enumerated by distance class s in 0..64: class s, lane p ->
    {p, (p+s)%128}; host ships 16 rotated copies of x^T (fp16)
  - 65 class products via 8 grouped DVE tensor_mul ops; 66 K=128 matmuls
    (2-way column-tiled) accumulate into PSUM [64+64, 512]
'p' (pencil) tiles - congruence-pencil squared projections (ACT-heavy):
  - outputs paired; for each pair (S_a, S_b) of quadratic forms, a real
    congruence basis B gives S_a = B^T D_a B, S_b = B^T D_b B (2x2 blocks
    from complex pencil eigenvalues handled with one extra (y1+y2)
    projection) => out = sum_k g_k (v_k . x)^2, <=192 projections/pencil
  - 48 projection matmuls [128x128] -> PSUM; ACT Square evacuates to fp16
    z2 in SBUF; contraction: 1 linear matmul + 2 matmuls/pencil (K=128 +
    K=64) into a 4-way column-tiled PSUM accumulator
  - fp16 throughout (bf16 fails: the pencil basis amplifies quantization
    ~40x; fp16 measured max rel err ~1.1e-2 vs gate 2e-2)

This splits the elementwise work (the bottleneck) between DVE (class
products) and ACT (pencil squares); GpSimd is avoided for compute (SBUF
port contention with DVE measured a 1.55x slowdown).

TPB instructions have a single sync-wait slot, but Tile emits multiple
waits on slot-recycling instructions; _split_multiwaits() post-processes
the BIR, hoisting extra waits onto injected same-engine NOPs.
"""

import os

import numpy as np
import ml_dtypes

INPUT_DIM = 128
OUTPUT_DIM = 64
BATCH = 32768
N_CORES = 8
B_CORE = BATCH // N_CORES  # 4096
TILE_B = 512
N_TILES = B_CORE // TILE_B  # 8

TILE_MODE = os.environ.get("K_TILE_MODE", "cpcpcpcc")
assert len(TILE_MODE) == N_TILES and set(TILE_MODE) <= {"c", "p"}
N_C = TILE_MODE.count("c")
N_P = TILE_MODE.count("p")

# every k-th 2-block square goes to DVE instead of ACT (0 = all ACT)
SQ_DVE = int(os.environ.get("K_SQ_DVE", "0"))

ROT_SET = [0, 1, 2, 3, 4, 5, 6, 7, 8, 16, 24, 32, 40, 48, 56, 64]
N_ROT = len(ROT_SET)
ROT_IDX = {d: i for i, d in enumerate(ROT_SET)}

N_PENCIL = OUTPUT_DIM // 2  # 32
R_PAD = 192                 # max 128 + 64 (all-complex pencil) exactly fits
N_PROJ = N_PENCIL * R_PAD   # 6144
N_PBLK = N_PROJ // 128      # 48 projection matmul blocks


def _class_ops():
    """(a, b) rotation pair per distance class s=0..64 with b - a = s."""
    ops = []
    for s in range(65):
        if s <= 8:
            a, b = 0, s
        else:
            k = (s - 1) // 8  # 1..7
            anchor = 8 * k + 8
            a, b = anchor - s, anchor
        assert a in ROT_SET and b in ROT_SET and b - a == s, (s, a, b)
        ops.append((a, b))
    return ops


CLASS_OPS = _class_ops()


# ---------------------------------------------------------------------------
# host: class-path weight packing (identical to the class-only kernel)
# ---------------------------------------------------------------------------

def _build_device_weights(W, b):
    """Permute W [64, 8384] into the class K-block layout [128, 66*64]."""
    W = np.asarray(W, np.float32)
    n = INPUT_DIM
    pair_off = {}
    c = 0
    for i in range(n):
        for j in range(i, n):
            pair_off[(i, j)] = c
            c += 1
    assert c == 8256

    Wd = np.zeros((66, 128, OUTPUT_DIM), np.float32)
    Wd[0] = W[:, 0:128].T  # linear block
    seen = set()
    for s in range(65):
        a, _bb = CLASS_OPS[s]
        for p in range(128):
            u = (p + a) % 128
            v = (p + a + s) % 128
            i, j = (u, v) if u <= v else (v, u)
            if (i, j) in seen:
                continue  # duplicate lane (s=64 second half)
            seen.add((i, j))
            Wd[1 + s, p] = W[:, 128 + pair_off[(i, j)]]
    assert len(seen) == 8256, len(seen)
    w_packed = np.ascontiguousarray(
        Wd.transpose(1, 0, 2).reshape(128, 66 * OUTPUT_DIM)
    ).astype(np.float16)
    return w_packed, np.asarray(b, np.float32)


# ---------------------------------------------------------------------------
# host: pencil decomposition
# ---------------------------------------------------------------------------

def _build_S(W2):
    """W2 [64, 8256] -> S [64,128,128] symmetric with x^T S_o x = sum W2 x_i x_j."""
    n = INPUT_DIM
    iu = np.triu_indices(n)
    S = np.zeros((OUTPUT_DIM, n, n))
    for o in range(OUTPUT_DIM):
        M = np.zeros((n, n))
        M[iu] = W2[o]
        S[o] = (M + M.T) / 2
    return S


def _pencil_decompose(Sa, Sb):
    """V [R,128] (unit rows), ga, gb [R]: x^T Sa x = sum ga_k (V_k.x)^2 etc."""
    n = Sa.shape[0]
    M = np.linalg.solve(Sb, Sa)
    lam, Vc = np.linalg.eig(M)
    cols = []
    used = np.zeros(n, bool)
    for i in range(n):
        if used[i]:
            continue
        if abs(lam[i].imag) < 1e-9 * max(1.0, abs(lam[i].real)):
            cols.append(Vc[:, i].real)
            used[i] = True
        else:
            rest = [k for k in range(i + 1, n) if not used[k]]
            j = min(rest, key=lambda k: abs(lam[k] - lam[i].conjugate()))
            cols.append(Vc[:, i].real)
            cols.append(Vc[:, i].imag)
            used[i] = used[j] = True
    X = np.stack(cols, axis=1)
    A = X.T @ Sa @ X
    Bm = X.T @ Sb @ X
    Vrows = np.linalg.inv(X)
    proj, ga, gb = [], [], []
    scale_a = np.abs(A).max()
    k = 0
    while k < n:
        if k + 1 < n and (abs(A[k, k + 1]) > 1e-8 * scale_a
                          or abs(Bm[k, k + 1]) > 1e-8 * scale_a):
            A2 = A[k:k + 2, k:k + 2]
            B2 = Bm[k:k + 2, k:k + 2]
            w, R = np.linalg.eigh(A2)  # rotate to diagonalize the A block
            B2r = R.T @ B2 @ R
            r1 = R[0, 0] * Vrows[k] + R[1, 0] * Vrows[k + 1]
            r2 = R[0, 1] * Vrows[k] + R[1, 1] * Vrows[k + 1]
            b12 = B2r[0, 1]
            # 2 b12 y1 y2 = b12[(y1+y2)^2 - y1^2 - y2^2]
            proj += [r1, r2, r1 + r2]
            ga += [w[0], w[1], 0.0]
            gb += [B2r[0, 0] - b12, B2r[1, 1] - b12, b12]
            k += 2
        else:
            proj.append(Vrows[k])
            ga.append(A[k, k])
            gb.append(Bm[k, k])
            k += 1
    V = np.stack(proj, axis=0)
    ga = np.asarray(ga)
    gb = np.asarray(gb)
    nrm = np.linalg.norm(V, axis=1)
    V = V / nrm[:, None]
    return V, ga * nrm**2, gb * nrm**2


def _build_pencil_weights(W, b):
    """Pack pencil projection/contraction tensors.

    Returns Vd [128, N_PROJ] fp16 (lhsT: feature x flat-proj) and
    Gd [128, N_PBLK*64] fp16: dense contraction stationaries - block j
    holds coefficient rows for flat projections [128j, 128j+128) across
    all 64 outputs (natural output order), and W1n [128, 64] fp16.
    """
    W = np.asarray(W, np.float64)
    W1, W2 = W[:, :128], W[:, 128:]
    S = _build_S(W2)

    Vflat = np.zeros((N_PROJ, 128))
    Gflat = np.zeros((N_PROJ, OUTPUT_DIM))
    for p in range(N_PENCIL):
        V, ga, gb = _pencil_decompose(S[2 * p], S[2 * p + 1])
        R = V.shape[0]
        assert R <= R_PAD, R
        Vflat[R_PAD * p:R_PAD * p + R] = V
        Gflat[R_PAD * p:R_PAD * p + R, 2 * p] = ga
        Gflat[R_PAD * p:R_PAD * p + R, 2 * p + 1] = gb

    Vd = np.ascontiguousarray(Vflat.T).astype(np.float16)
    Gd = np.ascontiguousarray(
        Gflat.reshape(N_PBLK, 128, OUTPUT_DIM).transpose(1, 0, 2).reshape(
            128, N_PBLK * OUTPUT_DIM
        )
    ).astype(np.float16)
    W1n = np.ascontiguousarray(W1.T).astype(np.float16)
    return Vd, Gd, W1n


def _split_multiwaits(nc, mybir):
    """TPB instructions have one sync-wait slot; hoist extras onto NOPs."""
    import bass_rust

    n_split = 0
    for fn in nc.m.functions:
        for bb in fn.blocks:
            out = []
            changed = False
            for inst in bb.instructions:
                si = getattr(inst, "sync_info", None)
                if si is not None and si.on_wait and len(si.on_wait) > 1:
                    for w in si.on_wait[:-1]:
                        n_split += 1
                        nop = bass_rust.InstNoOp(
                            name=f"I-mw{n_split}",
                            engine=inst.engine,
                            ins=[],
                            outs=[],
                            sync_info=mybir.SyncInfo(on_wait=[w], on_update=[]),
                            bass_nofuse=True,
                        )
                        out.append(nop)
                    inst.sync_info = mybir.SyncInfo(
                        on_wait=[si.on_wait[-1]], on_update=si.on_update
                    )
                    changed = True
                out.append(inst)
            if changed:
                bb.instructions = out
    return n_split


def build(x, W, b):
    """Build the Bass graph and per-core input maps. Returns (nc, in_maps)."""
    import concourse.bass as bass
    import concourse.mybir as mybir
    from concourse import tile

    f16 = mybir.dt.float16
    f32 = mybir.dt.float32

    # ---- host preprocessing ----
    xT = np.ascontiguousarray(np.asarray(x, np.float32).T).astype(np.float16)
    # xall[p, i, n] = feature (p + ROT_SET[i]) % 128 of sample n
    xall = np.stack([np.roll(xT, -d, axis=0) for d in ROT_SET], axis=1)
    w_packed, bias = _build_device_weights(W, b)
    Vd, Gd, W1n = _build_pencil_weights(W, b)

    # ---- device graph ----
    nc = bass.Bass()
    if N_C:
        xc_in = nc.declare_dram_parameter(
            "xallc", [N_C, 128, N_ROT, TILE_B], f16, isOutput=False
        )
        outc_ext = nc.declare_dram_parameter(
            "outc", [OUTPUT_DIM, N_C * TILE_B], f32, isOutput=True
        )
    if N_P:
        xp_in = nc.declare_dram_parameter(
            "xpen", [N_P, 128, TILE_B], f16, isOutput=False
        )
        v_in = nc.declare_dram_parameter("Vd", [128, N_PROJ], f16, isOutput=False)
        g_in = nc.declare_dram_parameter(
            "Gd", [128, N_PBLK * OUTPUT_DIM], f16, isOutput=False
        )
        w1n_in = nc.declare_dram_parameter("W1n", [128, 64], f16, isOutput=False)
        outp_ext = nc.declare_dram_parameter(
            "outp", [OUTPUT_DIM, N_P * TILE_B], f32, isOutput=True
        )
    w_in = nc.declare_dram_parameter("Wd", [128, 66 * 64], f16, isOutput=False)
    b_in = nc.declare_dram_parameter("bias", [OUTPUT_DIM, 1], f32, isOutput=False)

    # multi-class ops: one per anchor family, constant-stride rotation APs
    MC_OPS = [list(range(0, 9))] + [
        list(range(8 * k + 1, 8 * k + 9)) for k in range(1, 8)
    ]

    def rot_group_ap(xrt, classes):
        """[128, len(classes), TILE_B] APs (in0, in1)."""
        m = len(classes)
        us = [ROT_IDX[CLASS_OPS[s][0]] for s in classes]
        vs = [ROT_IDX[CLASS_OPS[s][1]] for s in classes]

        def mk(idx):
            if all(i == idx[0] for i in idx):
                return xrt[:, idx[0]: idx[0] + 1, :].to_broadcast(
                    [128, m, TILE_B]
                )
            d = idx[1] - idx[0]
            assert all(idx[j + 1] - idx[j] == d for j in range(m - 1)), idx
            return xrt[:, idx[0]:: d, :][:, 0:m, :]

        return mk(us), mk(vs)

    with tile.TileContext(nc) as tc:
        with (
            tc.tile_pool(name="consts", bufs=1) as consts,
            tc.tile_pool(name="xc", bufs=3) as xcp,
            tc.tile_pool(name="xp", bufs=2) as xpp,
            tc.tile_pool(name="prod", bufs=3) as prodp,
            tc.tile_pool(name="z2", bufs=(2 if N_C == 0 else 1)) as z2p,
            tc.tile_pool(name="outp", bufs=3) as outp,
            tc.tile_pool(name="proj", bufs=3, space="PSUM") as projp,
            tc.tile_pool(name="acc", bufs=2, space="PSUM") as accp,
        ):
            # contraction consts go on the GpSimd (SWDGE) queue; V rides
            # early on the fast sync queue (it gates the first projections)
            if N_P:
                v_sb = consts.tile([128, N_PROJ], f16)
                g_sb = consts.tile([128, N_PBLK * OUTPUT_DIM], f16)
                nc.gpsimd.dma_start(g_sb[:], g_in[:])
                w1n_sb = consts.tile([128, 64], f16)
                nc.gpsimd.dma_start(w1n_sb[:], w1n_in[:])
            w_sb = consts.tile([128, 66 * 64], f16)
            b_sb = consts.tile([OUTPUT_DIM, 1], f32)

            loaded = {}

            def load_input(idx, mode):
                if mode == "c":
                    xt = xcp.tile([128, N_ROT, TILE_B], f16, tag="xc", name="xc_t")
                    nc.sync.dma_start(xt[:], xc_in[idx][:])
                else:
                    xt = xpp.tile([128, TILE_B], f16, tag="xp", name="xp_t")
                    nc.sync.dma_start(xt[:], xp_in[idx][:])
                loaded[(mode, idx)] = xt

            def class_tile(xrt, ic):
                # acc halves: even classes + linear -> partitions 0:64,
                # odd classes -> partitions 64:128
                acc = accp.tile([128, TILE_B], f32, name="acc")
                nc.tensor.matmul(
                    acc[0:64, :],
                    w_sb[:, 0:64],
                    xrt[:, 0, :],
                    start=True,
                    stop=False,
                    tile_position=(0, 0),
                )
                first_odd = True
                for k, classes in enumerate(MC_OPS):
                    m = len(classes)
                    p_t = prodp.tile(
                        [128, m, TILE_B], f16, tag="prod" + str(m), name="p_t"
                    )
                    in0, in1 = rot_group_ap(xrt, classes)
                    nc.vector.tensor_mul(p_t[:], in0, in1)
                    for j, s in enumerate(classes):
                        half = s % 2
                        blk = 1 + s
                        nc.tensor.matmul(
                            acc[64 * half: 64 * half + 64, :],
                            w_sb[:, blk * 64: (blk + 1) * 64],
                            p_t[:, j, :],
                            start=(half == 1 and first_odd),
                            stop=(s == 64 or s == 63),
                            tile_position=(0, 64 * half),
                        )
                        if half == 1:
                            first_odd = False

                # ACT evacuates both PSUM halves; accumulating DMA adds the
                # odd half into DRAM (keeps DVE free for products)
                o_t = outp.tile([OUTPUT_DIM, TILE_B], f32, tag="o", name="o_t")
                o2_t = outp.tile([OUTPUT_DIM, TILE_B], f32, tag="o2", name="o2_t")
                nc.scalar.activation(
                    o_t[:],
                    acc[0:64, :],
                    mybir.ActivationFunctionType.Identity,
                    bias=b_sb[:, 0:1],
                )
                nc.scalar.copy(o2_t[:], acc[64:128, :])
                bs = slice(ic * TILE_B, (ic + 1) * TILE_B)
                nc.sync.dma_start(outc_ext[:, bs], o_t[:])
                nc.gpsimd.dma_start(
                    outc_ext[:, bs], o2_t[:], accum_op=mybir.AluOpType.add
                )

            def pencil_projections(x_t):
                # 48 projection matmuls -> PSUM pairs -> Square -> z2 (fp16)
                z2 = z2p.tile([128, N_PBLK, TILE_B], f16, tag="z2", name="z2_t")
                for jj in range(N_PBLK // 2):
                    pj = projp.tile([128, 2, TILE_B], f32, name="pj")
                    for h in range(2):
                        j = 2 * jj + h
                        nc.tensor.matmul(
                            pj[:, h, :],
                            v_sb[:, j * 128: (j + 1) * 128],
                            x_t[:],
                            start=True,
                            stop=True,
                        )
                    dst = z2[:, 2 * jj: 2 * jj + 2, :]
                    if SQ_DVE and jj % SQ_DVE == SQ_DVE - 1:
                        nc.vector.tensor_mul(dst, pj[:], pj[:])
                    else:
                        nc.scalar.activation(
                            dst, pj[:], mybir.ActivationFunctionType.Square
                        )
                return z2

            def pencil_contraction(z2, x_t, ip):
                # dense contraction: linear + one [128K x 64N] matmul per
                # z2 block, 2-way column-tiled (even j -> psum rows 0:64,
                # odd j -> 64:128), mirroring the class-tile structure
                acc = accp.tile([128, TILE_B], f32, name="acc")
                nc.tensor.matmul(
                    acc[0:64, :], w1n_sb[:], x_t[:], start=True, stop=False,
                    tile_position=(0, 0),
                )
                for j in range(N_PBLK):
                    half = j % 2
                    nc.tensor.matmul(
                        acc[64 * half: 64 * half + 64, :],
                        g_sb[:, j * 64: (j + 1) * 64],
                        z2[:, j, :],
                        start=(j == 1),
                        stop=(j >= N_PBLK - 2),
                        tile_position=(0, 64 * half),
                    )

                o_t = outp.tile([OUTPUT_DIM, TILE_B], f32, tag="o", name="o_t")
                o2_t = outp.tile([OUTPUT_DIM, TILE_B], f32, tag="o2", name="o2_t")
                nc.scalar.activation(
                    o_t[:],
                    acc[0:64, :],
                    mybir.ActivationFunctionType.Identity,
                    bias=b_sb[:, 0:1],
                )
                nc.scalar.copy(o2_t[:], acc[64:128, :])
                bs = slice(ip * TILE_B, (ip + 1) * TILE_B)
                nc.sync.dma_start(outp_ext[:, bs], o_t[:])
                nc.gpsimd.dma_start(
                    outp_ext[:, bs], o2_t[:], accum_op=mybir.AluOpType.add
                )

            # emission schedule: each pencil tile paired with a class tile.
            # Per pair: [projections + squares][class products + matmuls]
            # [pencil contraction][evacuations] - DVE chews class products
            # while ACT chews pencil squares, tensor serves both.
            units = []       # ('pair', ip, ic) | ('c', ic) | ('p', ip)
            n_pair = min(N_P, N_C)
            for i in range(n_pair):
                units.append(("pair", i, i))
            units += [("c", i) for i in range(n_pair, N_C)]
            units += [("p", i) for i in range(n_pair, N_P)]

            load_seq = []    # (mode, idx) in consumption order
            for u in units:
                if u[0] == "pair":
                    load_seq.append(("p", u[1]))
                    load_seq.append(("c", u[2]))
                else:
                    load_seq.append((u[0], u[1]))

            # first two inputs, then the class weights, then the rest ahead
            nload = 0

            def prefetch(n):
                nonlocal nload
                while nload < min(n, len(load_seq)):
                    load_input(load_seq[nload][1], load_seq[nload][0])
                    nload += 1

            prefetch(1)
            if N_P:
                nc.sync.dma_start(v_sb[:], v_in[:])
            prefetch(2)
            nc.sync.dma_start(w_sb[:], w_in[:])
            nc.sync.dma_start(b_sb[:], b_in[:])
            prefetch(4)

            done = 0
            for u in units:
                if u[0] == "pair":
                    _, ip, ic = u
                    z2 = pencil_projections(loaded[("p", ip)])
                    class_tile(loaded[("c", ic)], ic)
                    pencil_contraction(z2, loaded[("p", ip)], ip)
                    done += 2
                elif u[0] == "c":
                    class_tile(loaded[("c", u[1])], u[1])
                    done += 1
                else:
                    z2 = pencil_projections(loaded[("p", u[1])])
                    pencil_contraction(z2, loaded[("p", u[1])], u[1])
                    done += 1
                prefetch(done + 3)

    _split_multiwaits(nc, mybir)

    # ---- per-core input maps ----
    c_tiles = [t for t in range(N_TILES) if TILE_MODE[t] == "c"]
    p_tiles = [t for t in range(N_TILES) if TILE_MODE[t] == "p"]
    in_maps = []
    for c in range(N_CORES):
        base = c * B_CORE
        m = {"Wd": w_packed, "bias": bias.reshape(OUTPUT_DIM, 1)}
        if N_C:
            xc = np.stack([
                xall[:, :, base + t * TILE_B: base + (t + 1) * TILE_B]
                for t in c_tiles
            ])  # [N_C, 128, 16, TILE_B]
            m["xallc"] = np.ascontiguousarray(xc)
        if N_P:
            xp = np.stack([
                xT[:, base + t * TILE_B: base + (t + 1) * TILE_B]
                for t in p_tiles
            ])  # [N_P, 128, TILE_B]
            m["xpen"] = np.ascontiguousarray(xp)
            m["Vd"] = Vd
            m["Gd"] = Gd
            m["W1n"] = W1n
        in_maps.append(m)
    return nc, in_maps


def kernel(x, W, b, indices_0, indices_1):
    from concourse.bass_utils import run_bass_kernel_spmd

    nc, in_maps = build(x, W, b)
    res = run_bass_kernel_spmd(nc, in_maps, list(range(N_CORES))).results

    c_tiles = [t for t in range(N_TILES) if TILE_MODE[t] == "c"]
    p_tiles = [t for t in range(N_TILES) if TILE_MODE[t] == "p"]

    out = np.empty((BATCH, OUTPUT_DIM), np.float32)
    for c in range(N_CORES):
        base = c * B_CORE
        for name, tiles in (("outc", c_tiles), ("outp", p_tiles)):
            if not tiles:
                continue
            res_t = np.asarray(res[c][name], np.float32)  # [64, n*512]
            for i, t in enumerate(tiles):
                blk = res_t[:, i * TILE_B: (i + 1) * TILE_B]
                out[base + t * TILE_B: base + (t + 1) * TILE_B] = blk.T
    return out


# revision 22
# speedup vs baseline: 1.8258x; 1.0183x over previous
"""Polynomial features (degree 2) + linear layer, distributed over 8 TRN2 cores.

reference: A = [x, {x_i*x_j for i<=j}] (8384 coeffs); out = A @ W.T + b.

Hybrid kernel: each core processes 8 batch tiles of 512; each tile runs one
of two algorithms (TILE_MODE string, 'c'/'p'):

'c' (class) tiles - circular-distance-class products (DVE-bound):
  - pairs enumerated by distance class s in 0..64: class s, lane p ->
    {p, (p+s)%128}; host ships 16 rotated copies of x^T (fp16)
  - 65 class products via 8 grouped DVE tensor_mul ops; 66 K=128 matmuls
    (2-way column-tiled) accumulate into PSUM [64+64, 512]
'p' (pencil) tiles - congruence-pencil squared projections (ACT-heavy):
  - outputs paired; for each pair (S_a, S_b) of quadratic forms, a real
    congruence basis B gives S_a = B^T D_a B, S_b = B^T D_b B (2x2 blocks
    from complex pencil eigenvalues handled with one extra (y1+y2)
    projection) => out = sum_k g_k (v_k . x)^2, <=192 projections/pencil
  - 48 projection matmuls [128x128] -> PSUM; ACT Square evacuates to fp16
    z2 in SBUF; contraction: 1 linear matmul + 2 matmuls/pencil (K=128 +
    K=64) into a 4-way column-tiled PSUM accumulator
  - fp16 throughout (bf16 fails: the pencil basis amplifies quantization
    ~40x; fp16 measured max rel err ~1.1e-2 vs gate 2e-2)

This splits the elementwise work (the bottleneck) between DVE (class
products) and ACT (pencil squares); GpSimd is avoided for compute (SBUF
port contention with DVE measured a 1.55x slowdown).

TPB instructions have a single sync-wait slot, but Tile emits multiple
waits on slot-recycling instructions; _split_multiwaits() post-processes
the BIR, hoisting extra waits onto injected same-engine NOPs.
"""

import os

import numpy as np
import ml_dtypes

INPUT_DIM = 128
OUTPUT_DIM = 64
BATCH = 32768
N_CORES = 8
B_CORE = BATCH // N_CORES  # 4096
TILE_B = 512
N_TILES = B_CORE // TILE_B  # 8

TILE_MODE = os.environ.get("K_TILE_MODE", "cpcpcpcc")
assert len(TILE_MODE) == N_TILES and set(TILE_MODE) <= {"c", "p"}
N_C = TILE_MODE.count("c")
N_P = TILE_MODE.count("p")

# every k-th 2-block square goes to DVE instead of ACT (0 = all ACT)
SQ_DVE = int(os.environ.get("K_SQ_DVE", "0"))

ROT_SET = [0, 1, 2, 3, 4, 5, 6, 7, 8, 16, 24, 32, 40, 48, 56, 64]
N_ROT = len(ROT_SET)
ROT_IDX = {d: i for i, d in enumerate(ROT_SET)}

N_PENCIL = OUTPUT_DIM // 2  # 32
R_PAD = 192                 # max 128 + 64 (all-complex pencil) exactly fits
N_PROJ = N_PENCIL * R_PAD   # 6144
N_PBLK = N_PROJ // 128      # 48 projection matmul blocks


def _class_ops():
    """(a, b) rotation pair per distance class s=0..64 with b - a = s."""
    ops = []
    for s in range(65):
        if s <= 8:
            a, b = 0, s
        else:
            k = (s - 1) // 8  # 1..7
            anchor = 8 * k + 8
            a, b = anchor - s, anchor
        assert a in ROT_SET and b in ROT_SET and b - a == s, (s, a, b)
        ops.append((a, b))
    return ops


CLASS_OPS = _class_ops()


# ---------------------------------------------------------------------------
# host: class-path weight packing (identical to the class-only kernel)
# ---------------------------------------------------------------------------

def _build_device_weights(W, b):
    """Permute W [64, 8384] into the class K-block layout [128, 66*64]."""
    W = np.asarray(W, np.float32)
    n = INPUT_DIM
    pair_off = {}
    c = 0
    for i in range(n):
        for j in range(i, n):
            pair_off[(i, j)] = c
            c += 1
    assert c == 8256

    Wd = np.zeros((66, 128, OUTPUT_DIM), np.float32)
    Wd[0] = W[:, 0:128].T  # linear block
    seen = set()
    for s in range(65):
        a, _bb = CLASS_OPS[s]
        for p in range(128):
            u = (p + a) % 128
            v = (p + a + s) % 128
            i, j = (u, v) if u <= v else (v, u)
            if (i, j) in seen:
                continue  # duplicate lane (s=64 second half)
            seen.add((i, j))
            Wd[1 + s, p] = W[:, 128 + pair_off[(i, j)]]
    assert len(seen) == 8256, len(seen)
    w_packed = np.ascontiguousarray(
        Wd.transpose(1, 0, 2).reshape(128, 66 * OUTPUT_DIM)
    ).astype(np.float16)
    return w_packed, np.asarray(b, np.float32)


# ---------------------------------------------------------------------------
# host: pencil decomposition
# ---------------------------------------------------------------------------

def _build_S(W2):
    """W2 [64, 8256] -> S [64,128,128] symmetric with x^T S_o x = sum W2 x_i x_j."""
    n = INPUT_DIM
    iu = np.triu_indices(n)
    S = np.zeros((OUTPUT_DIM, n, n))
    for o in range(OUTPUT_DIM):
        M = np.zeros((n, n))
        M[iu] = W2[o]
        S[o] = (M + M.T) / 2
    return S


def _pencil_decompose(Sa, Sb):
    """V [R,128] (unit rows), ga, gb [R]: x^T Sa x = sum ga_k (V_k.x)^2 etc."""
    n = Sa.shape[0]
    M = np.linalg.solve(Sb, Sa)
    lam, Vc = np.linalg.eig(M)
    cols = []
    used = np.zeros(n, bool)
    for i in range(n):
        if used[i]:
            continue
        if abs(lam[i].imag) < 1e-9 * max(1.0, abs(lam[i].real)):
            cols.append(Vc[:, i].real)
            used[i] = True
        else:
            rest = [k for k in range(i + 1, n) if not used[k]]
            j = min(rest, key=lambda k: abs(lam[k] - lam[i].conjugate()))
            cols.append(Vc[:, i].real)
            cols.append(Vc[:, i].imag)
            used[i] = used[j] = True
    X = np.stack(cols, axis=1)
    A = X.T @ Sa @ X
    Bm = X.T @ Sb @ X
    Vrows = np.linalg.inv(X)
    proj, ga, gb = [], [], []
    scale_a = np.abs(A).max()
    k = 0
    while k < n:
        if k + 1 < n and (abs(A[k, k + 1]) > 1e-8 * scale_a
                          or abs(Bm[k, k + 1]) > 1e-8 * scale_a):
            A2 = A[k:k + 2, k:k + 2]
            B2 = Bm[k:k + 2, k:k + 2]
            w, R = np.linalg.eigh(A2)  # rotate to diagonalize the A block
            B2r = R.T @ B2 @ R
            r1 = R[0, 0] * Vrows[k] + R[1, 0] * Vrows[k + 1]
            r2 = R[0, 1] * Vrows[k] + R[1, 1] * Vrows[k + 1]
            b12 = B2r[0, 1]
            # 2 b12 y1 y2 = b12[(y1+y2)^2 - y1^2 - y2^2]
            proj += [r1, r2, r1 + r2]
            ga += [w[0], w[1], 0.0]
            gb += [B2r[0, 0] - b12, B2r[1, 1] - b12, b12]
            k += 2
        else:
            proj.append(Vrows[k])
            ga.append(A[k, k])
            gb.append(Bm[k, k])
            k += 1
    V = np.stack(proj, axis=0)
    ga = np.asarray(ga)
    gb = np.asarray(gb)
    nrm = np.linalg.norm(V, axis=1)
    V = V / nrm[:, None]
    return V, ga * nrm**2, gb * nrm**2


def _build_pencil_weights(W, b):
    """Pack pencil projection/contraction tensors.

    Returns Vd [128, N_PROJ] fp16 (lhsT: feature x flat-proj) and
    Gd [128, N_PBLK*64] fp16: dense contraction stationaries - block j
    holds coefficient rows for flat projections [128j, 128j+128) across
    all 64 outputs (natural output order), and W1n [128, 64] fp16.
    """
    W = np.asarray(W, np.float64)
    W1, W2 = W[:, :128], W[:, 128:]
    S = _build_S(W2)

    Vflat = np.zeros((N_PROJ, 128))
    Gflat = np.zeros((N_PROJ, OUTPUT_DIM))
    for p in range(N_PENCIL):
        V, ga, gb = _pencil_decompose(S[2 * p], S[2 * p + 1])
        R = V.shape[0]
        assert R <= R_PAD, R
        Vflat[R_PAD * p:R_PAD * p + R] = V
        Gflat[R_PAD * p:R_PAD * p + R, 2 * p] = ga
        Gflat[R_PAD * p:R_PAD * p + R, 2 * p + 1] = gb

    Vd = np.ascontiguousarray(Vflat.T).astype(np.float16)
    Gd = np.ascontiguousarray(
        Gflat.reshape(N_PBLK, 128, OUTPUT_DIM).transpose(1, 0, 2).reshape(
            128, N_PBLK * OUTPUT_DIM
        )
    ).astype(np.float16)
    W1n = np.ascontiguousarray(W1.T).astype(np.float16)
    return Vd, Gd, W1n


def _split_multiwaits(nc, mybir):
    """TPB instructions have one sync-wait slot; hoist extras onto NOPs."""
    import bass_rust

    n_split = 0
    for fn in nc.m.functions:
        for bb in fn.blocks:
            out = []
            changed = False
            for inst in bb.instructions:
                si = getattr(inst, "sync_info", None)
                if si is not None and si.on_wait and len(si.on_wait) > 1:
                    for w in si.on_wait[:-1]:
                        n_split += 1
                        nop = bass_rust.InstNoOp(
                            name=f"I-mw{n_split}",
                            engine=inst.engine,
                            ins=[],
                            outs=[],
                            sync_info=mybir.SyncInfo(on_wait=[w], on_update=[]),
                            bass_nofuse=True,
                        )
                        out.append(nop)
                    inst.sync_info = mybir.SyncInfo(
                        on_wait=[si.on_wait[-1]], on_update=si.on_update
                    )
                    changed = True
                out.append(inst)
            if changed:
                bb.instructions = out
    return n_split


def build(x, W, b):
    """Build the Bass graph and per-core input maps. Returns (nc, in_maps)."""
    import concourse.bass as bass
    import concourse.mybir as mybir
    from concourse import tile

    f16 = mybir.dt.float16
    f32 = mybir.dt.float32

    # ---- host preprocessing ----
    xT = np.ascontiguousarray(np.asarray(x, np.float32).T).astype(np.float16)
    # xall[p, i, n] = feature (p + ROT_SET[i]) % 128 of sample n
    xall = np.stack([np.roll(xT, -d, axis=0) for d in ROT_SET], axis=1)
    w_packed, bias = _build_device_weights(W, b)
    Vd, Gd, W1n = _build_pencil_weights(W, b)

    # ---- device graph ----
    nc = bass.Bass()
    if N_C:
        xc_in = nc.declare_dram_parameter(
            "xallc", [N_C, 128, N_ROT, TILE_B], f16, isOutput=False
        )
        outc_ext = nc.declare_dram_parameter(
            "outc", [OUTPUT_DIM, N_C * TILE_B], f32, isOutput=True
        )
    if N_P:
        xp_in = nc.declare_dram_parameter(
            "xpen", [N_P, 128, TILE_B], f16, isOutput=False
        )
        v_in = nc.declare_dram_parameter("Vd", [128, N_PROJ], f16, isOutput=False)
        g_in = nc.declare_dram_parameter(
            "Gd", [128, N_PBLK * OUTPUT_DIM], f16, isOutput=False
        )
        w1n_in = nc.declare_dram_parameter("W1n", [128, 64], f16, isOutput=False)
        outp_ext = nc.declare_dram_parameter(
            "outp", [OUTPUT_DIM, N_P * TILE_B], f32, isOutput=True
        )
    w_in = nc.declare_dram_parameter("Wd", [128, 66 * 64], f16, isOutput=False)
    b_in = nc.declare_dram_parameter("bias", [OUTPUT_DIM, 1], f32, isOutput=False)

    # multi-class ops: one per anchor family, constant-stride rotation APs
    MC_OPS = [list(range(0, 9))] + [
        list(range(8 * k + 1, 8 * k + 9)) for k in range(1, 8)
    ]

    def rot_group_ap(xrt, classes):
        """[128, len(classes), TILE_B] APs (in0, in1)."""
        m = len(classes)
        us = [ROT_IDX[CLASS_OPS[s][0]] for s in classes]
        vs = [ROT_IDX[CLASS_OPS[s][1]] for s in classes]

        def mk(idx):
            if all(i == idx[0] for i in idx):
                return xrt[:, idx[0]: idx[0] + 1, :].to_broadcast(
                    [128, m, TILE_B]
                )
            d = idx[1] - idx[0]
            assert all(idx[j + 1] - idx[j] == d for j in range(m - 1)), idx
            return xrt[:, idx[0]:: d, :][:, 0:m, :]

        return mk(us), mk(vs)

    with tile.TileContext(nc) as tc:
        with (
            tc.tile_pool(name="consts", bufs=1) as consts,
            tc.tile_pool(name="xc", bufs=3) as xcp,
            tc.tile_pool(name="xp", bufs=2) as xpp,
            tc.tile_pool(name="prod", bufs=3) as prodp,
            tc.tile_pool(name="z2", bufs=(2 if N_C == 0 else 1)) as z2p,
            tc.tile_pool(name="outp", bufs=3) as outp,
            tc.tile_pool(name="proj", bufs=2, space="PSUM") as projp,
            tc.tile_pool(name="acc", bufs=2, space="PSUM") as accp,
        ):
            # contraction consts go on the GpSimd (SWDGE) queue; V rides
            # early on the fast sync queue (it gates the first projections)
            if N_P:
                v_sb = consts.tile([128, N_PROJ], f16)
                g_sb = consts.tile([128, N_PBLK * OUTPUT_DIM], f16)
                nc.gpsimd.dma_start(g_sb[:], g_in[:])
                w1n_sb = consts.tile([128, 64], f16)
                nc.gpsimd.dma_start(w1n_sb[:], w1n_in[:])
            w_sb = consts.tile([128, 66 * 64], f16)
            b_sb = consts.tile([OUTPUT_DIM, 1], f32)

            loaded = {}

            def load_input(idx, mode):
                if mode == "c":
                    xt = xcp.tile([128, N_ROT, TILE_B], f16, tag="xc", name="xc_t")
                    nc.sync.dma_start(xt[:], xc_in[idx][:])
                else:
                    xt = xpp.tile([128, TILE_B], f16, tag="xp", name="xp_t")
                    nc.sync.dma_start(xt[:], xp_in[idx][:])
                loaded[(mode, idx)] = xt

            def class_tile(xrt, ic):
                # acc halves: even classes + linear -> partitions 0:64,
                # odd classes -> partitions 64:128
                acc = accp.tile([128, TILE_B], f32, name="acc")
                nc.tensor.matmul(
                    acc[0:64, :],
                    w_sb[:, 0:64],
                    xrt[:, 0, :],
                    start=True,
                    stop=False,
                    tile_position=(0, 0),
                )
                first_odd = True
                for k, classes in enumerate(MC_OPS):
                    m = len(classes)
                    p_t = prodp.tile(
                        [128, m, TILE_B], f16, tag="prod" + str(m), name="p_t"
                    )
                    in0, in1 = rot_group_ap(xrt, classes)
                    nc.vector.tensor_mul(p_t[:], in0, in1)
                    for j, s in enumerate(classes):
                        half = s % 2
                        blk = 1 + s
                        nc.tensor.matmul(
                            acc[64 * half: 64 * half + 64, :],
                            w_sb[:, blk * 64: (blk + 1) * 64],
                            p_t[:, j, :],
                            start=(half == 1 and first_odd),
                            stop=(s == 64 or s == 63),
                            tile_position=(0, 64 * half),
                        )
                        if half == 1:
                            first_odd = False

                # ACT evacuates both PSUM halves; accumulating DMA adds the
                # odd half into DRAM (keeps DVE free for products)
                o_t = outp.tile([OUTPUT_DIM, TILE_B], f32, tag="o", name="o_t")
                o2_t = outp.tile([OUTPUT_DIM, TILE_B], f32, tag="o2", name="o2_t")
                nc.scalar.activation(
                    o_t[:],
                    acc[0:64, :],
                    mybir.ActivationFunctionType.Identity,
                    bias=b_sb[:, 0:1],
                )
                nc.scalar.copy(o2_t[:], acc[64:128, :])
                bs = slice(ic * TILE_B, (ic + 1) * TILE_B)
                nc.sync.dma_start(outc_ext[:, bs], o_t[:])
                nc.gpsimd.dma_start(
                    outc_ext[:, bs], o2_t[:], accum_op=mybir.AluOpType.add
                )

            def pencil_projections(x_t):
                # 48 projection matmuls -> PSUM triples -> Square -> z2 (fp16)
                z2 = z2p.tile([128, N_PBLK, TILE_B], f16, tag="z2", name="z2_t")
                for jj in range(N_PBLK // 3):
                    pj = projp.tile([128, 3, TILE_B], f32, name="pj")
                    for h in range(3):
                        j = 3 * jj + h
                        nc.tensor.matmul(
                            pj[:, h, :],
                            v_sb[:, j * 128: (j + 1) * 128],
                            x_t[:],
                            start=True,
                            stop=True,
                        )
                    dst = z2[:, 3 * jj: 3 * jj + 3, :]
                    if SQ_DVE and jj % SQ_DVE == SQ_DVE - 1:
                        nc.vector.tensor_mul(dst, pj[:], pj[:])
                    else:
                        nc.scalar.activation(
                            dst, pj[:], mybir.ActivationFunctionType.Square
                        )
                return z2

            def pencil_contraction(z2, x_t, ip):
                # dense contraction: linear + one [128K x 64N] matmul per
                # z2 block, 2-way column-tiled (even j -> psum rows 0:64,
                # odd j -> 64:128), mirroring the class-tile structure
                acc = accp.tile([128, TILE_B], f32, name="acc")
                nc.tensor.matmul(
                    acc[0:64, :], w1n_sb[:], x_t[:], start=True, stop=False,
                    tile_position=(0, 0),
                )
                for j in range(N_PBLK):
                    half = j % 2
                    nc.tensor.matmul(
                        acc[64 * half: 64 * half + 64, :],
                        g_sb[:, j * 64: (j + 1) * 64],
                        z2[:, j, :],
                        start=(j == 1),
                        stop=(j >= N_PBLK - 2),
                        tile_position=(0, 64 * half),
                    )

                o_t = outp.tile([OUTPUT_DIM, TILE_B], f32, tag="o", name="o_t")
                o2_t = outp.tile([OUTPUT_DIM, TILE_B], f32, tag="o2", name="o2_t")
                nc.scalar.activation(
                    o_t[:],
                    acc[0:64, :],
                    mybir.ActivationFunctionType.Identity,
                    bias=b_sb[:, 0:1],
                )
                nc.scalar.copy(o2_t[:], acc[64:128, :])
                bs = slice(ip * TILE_B, (ip + 1) * TILE_B)
                nc.sync.dma_start(outp_ext[:, bs], o_t[:])
                nc.gpsimd.dma_start(
                    outp_ext[:, bs], o2_t[:], accum_op=mybir.AluOpType.add
                )

            # emission schedule: 'A' = pencil projections+squares (feeds
            # ACT), 'B' = class products+matmuls then pencil contraction
            # (feeds DVE, tensor), 'c' = unpaired class tile. Unpaired
            # class tiles are interleaved between pairs so their DVE work
            # overlaps neighbouring pencil ACT work:
            #   A0 B0 A1 c A2 B1(c) ... pattern below: A_i emitted, then
            #   one B or lone-c keeps DVE fed while squares run.
            n_pair = min(N_P, N_C)
            lone_c = list(range(n_pair, N_C))
            lone_p = list(range(n_pair, N_P))
            units = []       # ('A', ip) | ('B', ip, ic) | ('c', ic) | ('p', ip)
            for i in range(n_pair):
                units.append(("A", i))
                if i > 0 and lone_c:
                    units.append(("c", lone_c.pop(0)))
                units.append(("B", i, i))
            units += [("c", i) for i in lone_c]
            for i in lone_p:
                units.append(("A", i))
                units.append(("B", i, None))

            load_seq = []    # (mode, idx) in consumption order
            for u in units:
                if u[0] == "A":
                    load_seq.append(("p", u[1]))
                elif u[0] == "B":
                    if u[2] is not None:
                        load_seq.append(("c", u[2]))
                elif u[0] == "c":
                    load_seq.append(("c", u[1]))

            # first two inputs, then the class weights, then the rest ahead
            nload = 0

            def prefetch(n):
                nonlocal nload
                while nload < min(n, len(load_seq)):
                    load_input(load_seq[nload][1], load_seq[nload][0])
                    nload += 1

            prefetch(1)
            if N_P:
                nc.sync.dma_start(v_sb[:, 0: N_PROJ // 2], v_in[:, 0: N_PROJ // 2])
                nc.sync.dma_start(v_sb[:, N_PROJ // 2:], v_in[:, N_PROJ // 2:])
            prefetch(2)
            nc.sync.dma_start(w_sb[:], w_in[:])
            nc.sync.dma_start(b_sb[:], b_in[:])
            prefetch(4)

            done = 0
            z2_of = {}
            for u in units:
                if u[0] == "A":
                    z2_of[u[1]] = pencil_projections(loaded[("p", u[1])])
                    done += 1
                elif u[0] == "B":
                    _, ip, ic = u
                    if ic is not None:
                        class_tile(loaded[("c", ic)], ic)
                        done += 1
                    pencil_contraction(z2_of.pop(ip), loaded[("p", ip)], ip)
                elif u[0] == "c":
                    class_tile(loaded[("c", u[1])], u[1])
                    done += 1
                prefetch(done + 3)

    _split_multiwaits(nc, mybir)

    # ---- per-core input maps ----
    c_tiles = [t for t in range(N_TILES) if TILE_MODE[t] == "c"]
    p_tiles = [t for t in range(N_TILES) if TILE_MODE[t] == "p"]
    in_maps = []
    for c in range(N_CORES):
        base = c * B_CORE
        m = {"Wd": w_packed, "bias": bias.reshape(OUTPUT_DIM, 1)}
        if N_C:
            xc = np.stack([
                xall[:, :, base + t * TILE_B: base + (t + 1) * TILE_B]
                for t in c_tiles
            ])  # [N_C, 128, 16, TILE_B]
            m["xallc"] = np.ascontiguousarray(xc)
        if N_P:
            xp = np.stack([
                xT[:, base + t * TILE_B: base + (t + 1) * TILE_B]
                for t in p_tiles
            ])  # [N_P, 128, TILE_B]
            m["xpen"] = np.ascontiguousarray(xp)
            m["Vd"] = Vd
            m["Gd"] = Gd
            m["W1n"] = W1n
        in_maps.append(m)
    return nc, in_maps


def kernel(x, W, b, indices_0, indices_1):
    from concourse.bass_utils import run_bass_kernel_spmd

    nc, in_maps = build(x, W, b)
    res = run_bass_kernel_spmd(nc, in_maps, list(range(N_CORES))).results

    c_tiles = [t for t in range(N_TILES) if TILE_MODE[t] == "c"]
    p_tiles = [t for t in range(N_TILES) if TILE_MODE[t] == "p"]

    out = np.empty((BATCH, OUTPUT_DIM), np.float32)
    for c in range(N_CORES):
        base = c * B_CORE
        for name, tiles in (("outc", c_tiles), ("outp", p_tiles)):
            if not tiles:
                continue
            res_t = np.asarray(res[c][name], np.float32)  # [64, n*512]
            for i, t in enumerate(tiles):
                blk = res_t[:, i * TILE_B: (i + 1) * TILE_B]
                out[base + t * TILE_B: base + (t + 1) * TILE_B] = blk.T
    return out


# revision 23
# speedup vs baseline: 1.8571x; 1.0171x over previous
"""Polynomial features (degree 2) + linear layer, distributed over 8 TRN2 cores.

reference: A = [x, {x_i*x_j for i<=j}] (8384 coeffs); out = A @ W.T + b.

Hybrid kernel: each core processes 8 batch tiles of 512; each tile runs one
of two algorithms (TILE_MODE string, 'c'/'p'):

'c' (class) tiles - circular-distance-class products (DVE-bound):
  - pairs enumerated by distance class s in 0..64: class s, lane p ->
    {p, (p+s)%128}; host ships 16 rotated copies of x^T (fp16)
  - 65 class products via 8 grouped DVE tensor_mul ops; 66 K=128 matmuls
    (2-way column-tiled) accumulate into PSUM [64+64, 512]
'p' (pencil) tiles - congruence-pencil squared projections (ACT-heavy):
  - outputs paired; for each pair (S_a, S_b) of quadratic forms, a real
    congruence basis B gives S_a = B^T D_a B, S_b = B^T D_b B (2x2 blocks
    from complex pencil eigenvalues handled with one extra (y1+y2)
    projection) => out = sum_k g_k (v_k . x)^2, <=192 projections/pencil
  - 48 projection matmuls [128x128] -> PSUM; ACT Square evacuates to fp16
    z2 in SBUF; contraction: 1 linear matmul + 2 matmuls/pencil (K=128 +
    K=64) into a 4-way column-tiled PSUM accumulator
  - fp16 throughout (bf16 fails: the pencil basis amplifies quantization
    ~40x; fp16 measured max rel err ~1.1e-2 vs gate 2e-2)

This splits the elementwise work (the bottleneck) between DVE (class
products) and ACT (pencil squares); GpSimd is avoided for compute (SBUF
port contention with DVE measured a 1.55x slowdown).

TPB instructions have a single sync-wait slot, but Tile emits multiple
waits on slot-recycling instructions; _split_multiwaits() post-processes
the BIR, hoisting extra waits onto injected same-engine NOPs.
"""

import os

import numpy as np
import ml_dtypes

INPUT_DIM = 128
OUTPUT_DIM = 64
BATCH = 32768
N_CORES = 8
B_CORE = BATCH // N_CORES  # 4096
TILE_B = 512
N_TILES = B_CORE // TILE_B  # 8

TILE_MODE = os.environ.get("K_TILE_MODE", "cpcpcpcc")
assert len(TILE_MODE) == N_TILES and set(TILE_MODE) <= {"c", "p"}
N_C = TILE_MODE.count("c")
N_P = TILE_MODE.count("p")

# every k-th 2-block square goes to DVE instead of ACT (0 = all ACT)
SQ_DVE = int(os.environ.get("K_SQ_DVE", "0"))

ROT_SET = [0, 1, 2, 3, 4, 5, 6, 7, 8, 16, 24, 32, 40, 48, 56, 64]
N_ROT = len(ROT_SET)
ROT_IDX = {d: i for i, d in enumerate(ROT_SET)}

N_PENCIL = OUTPUT_DIM // 2  # 32
R_PAD = 192                 # max 128 + 64 (all-complex pencil) exactly fits
N_PROJ = N_PENCIL * R_PAD   # 6144
N_PBLK = N_PROJ // 128      # 48 projection matmul blocks


def _class_ops():
    """(a, b) rotation pair per distance class s=0..64 with b - a = s."""
    ops = []
    for s in range(65):
        if s <= 8:
            a, b = 0, s
        else:
            k = (s - 1) // 8  # 1..7
            anchor = 8 * k + 8
            a, b = anchor - s, anchor
        assert a in ROT_SET and b in ROT_SET and b - a == s, (s, a, b)
        ops.append((a, b))
    return ops


CLASS_OPS = _class_ops()


# ---------------------------------------------------------------------------
# host: class-path weight packing (identical to the class-only kernel)
# ---------------------------------------------------------------------------

def _build_device_weights(W, b):
    """Permute W [64, 8384] into the class K-block layout [128, 66*64]."""
    W = np.asarray(W, np.float32)
    n = INPUT_DIM
    pair_off = {}
    c = 0
    for i in range(n):
        for j in range(i, n):
            pair_off[(i, j)] = c
            c += 1
    assert c == 8256

    Wd = np.zeros((66, 128, OUTPUT_DIM), np.float32)
    Wd[0] = W[:, 0:128].T  # linear block
    seen = set()
    for s in range(65):
        a, _bb = CLASS_OPS[s]
        for p in range(128):
            u = (p + a) % 128
            v = (p + a + s) % 128
            i, j = (u, v) if u <= v else (v, u)
            if (i, j) in seen:
                continue  # duplicate lane (s=64 second half)
            seen.add((i, j))
            Wd[1 + s, p] = W[:, 128 + pair_off[(i, j)]]
    assert len(seen) == 8256, len(seen)
    w_packed = np.ascontiguousarray(
        Wd.transpose(1, 0, 2).reshape(128, 66 * OUTPUT_DIM)
    ).astype(np.float16)
    return w_packed, np.asarray(b, np.float32)


# ---------------------------------------------------------------------------
# host: pencil decomposition
# ---------------------------------------------------------------------------

def _build_S(W2):
    """W2 [64, 8256] -> S [64,128,128] symmetric with x^T S_o x = sum W2 x_i x_j."""
    n = INPUT_DIM
    iu = np.triu_indices(n)
    S = np.zeros((OUTPUT_DIM, n, n))
    for o in range(OUTPUT_DIM):
        M = np.zeros((n, n))
        M[iu] = W2[o]
        S[o] = (M + M.T) / 2
    return S


def _pencil_decompose(Sa, Sb):
    """V [R,128] (unit rows), ga, gb [R]: x^T Sa x = sum ga_k (V_k.x)^2 etc."""
    n = Sa.shape[0]
    M = np.linalg.solve(Sb, Sa)
    lam, Vc = np.linalg.eig(M)
    cols = []
    used = np.zeros(n, bool)
    for i in range(n):
        if used[i]:
            continue
        if abs(lam[i].imag) < 1e-9 * max(1.0, abs(lam[i].real)):
            cols.append(Vc[:, i].real)
            used[i] = True
        else:
            rest = [k for k in range(i + 1, n) if not used[k]]
            j = min(rest, key=lambda k: abs(lam[k] - lam[i].conjugate()))
            cols.append(Vc[:, i].real)
            cols.append(Vc[:, i].imag)
            used[i] = used[j] = True
    X = np.stack(cols, axis=1)
    A = X.T @ Sa @ X
    Bm = X.T @ Sb @ X
    Vrows = np.linalg.inv(X)
    proj, ga, gb = [], [], []
    scale_a = np.abs(A).max()
    k = 0
    while k < n:
        if k + 1 < n and (abs(A[k, k + 1]) > 1e-8 * scale_a
                          or abs(Bm[k, k + 1]) > 1e-8 * scale_a):
            A2 = A[k:k + 2, k:k + 2]
            B2 = Bm[k:k + 2, k:k + 2]
            w, R = np.linalg.eigh(A2)  # rotate to diagonalize the A block
            B2r = R.T @ B2 @ R
            r1 = R[0, 0] * Vrows[k] + R[1, 0] * Vrows[k + 1]
            r2 = R[0, 1] * Vrows[k] + R[1, 1] * Vrows[k + 1]
            b12 = B2r[0, 1]
            # 2 b12 y1 y2 = b12[(y1+y2)^2 - y1^2 - y2^2]
            proj += [r1, r2, r1 + r2]
            ga += [w[0], w[1], 0.0]
            gb += [B2r[0, 0] - b12, B2r[1, 1] - b12, b12]
            k += 2
        else:
            proj.append(Vrows[k])
            ga.append(A[k, k])
            gb.append(Bm[k, k])
            k += 1
    V = np.stack(proj, axis=0)
    ga = np.asarray(ga)
    gb = np.asarray(gb)
    nrm = np.linalg.norm(V, axis=1)
    V = V / nrm[:, None]
    return V, ga * nrm**2, gb * nrm**2


def _build_pencil_weights(W, b):
    """Pack pencil projection/contraction tensors.

    Returns Vd [128, N_PROJ] fp16 (lhsT: feature x flat-proj) and
    Gd [128, N_PBLK*64] fp16: dense contraction stationaries - block j
    holds coefficient rows for flat projections [128j, 128j+128) across
    all 64 outputs (natural output order), and W1n [128, 64] fp16.
    """
    W = np.asarray(W, np.float64)
    W1, W2 = W[:, :128], W[:, 128:]
    S = _build_S(W2)

    Vflat = np.zeros((N_PROJ, 128))
    Gflat = np.zeros((N_PROJ, OUTPUT_DIM))
    for p in range(N_PENCIL):
        V, ga, gb = _pencil_decompose(S[2 * p], S[2 * p + 1])
        R = V.shape[0]
        assert R <= R_PAD, R
        Vflat[R_PAD * p:R_PAD * p + R] = V
        Gflat[R_PAD * p:R_PAD * p + R, 2 * p] = ga
        Gflat[R_PAD * p:R_PAD * p + R, 2 * p + 1] = gb

    Vd = np.ascontiguousarray(Vflat.T).astype(np.float16)
    Gd = np.ascontiguousarray(
        Gflat.reshape(N_PBLK, 128, OUTPUT_DIM).transpose(1, 0, 2).reshape(
            128, N_PBLK * OUTPUT_DIM
        )
    ).astype(np.float16)
    W1n = np.ascontiguousarray(W1.T).astype(np.float16)
    return Vd, Gd, W1n


def _split_multiwaits(nc, mybir):
    """TPB instructions have one sync-wait slot; hoist extras onto NOPs."""
    import bass_rust

    n_split = 0
    for fn in nc.m.functions:
        for bb in fn.blocks:
            out = []
            changed = False
            for inst in bb.instructions:
                si = getattr(inst, "sync_info", None)
                if si is not None and si.on_wait and len(si.on_wait) > 1:
                    for w in si.on_wait[:-1]:
                        n_split += 1
                        nop = bass_rust.InstNoOp(
                            name=f"I-mw{n_split}",
                            engine=inst.engine,
                            ins=[],
                            outs=[],
                            sync_info=mybir.SyncInfo(on_wait=[w], on_update=[]),
                            bass_nofuse=True,
                        )
                        out.append(nop)
                    inst.sync_info = mybir.SyncInfo(
                        on_wait=[si.on_wait[-1]], on_update=si.on_update
                    )
                    changed = True
                out.append(inst)
            if changed:
                bb.instructions = out
    return n_split


def build(x, W, b):
    """Build the Bass graph and per-core input maps. Returns (nc, in_maps)."""
    import concourse.bass as bass
    import concourse.mybir as mybir
    from concourse import tile

    f16 = mybir.dt.float16
    f32 = mybir.dt.float32

    # ---- host preprocessing ----
    xT = np.ascontiguousarray(np.asarray(x, np.float32).T).astype(np.float16)
    # xall[p, i, n] = feature (p + ROT_SET[i]) % 128 of sample n
    xall = np.stack([np.roll(xT, -d, axis=0) for d in ROT_SET], axis=1)
    w_packed, bias = _build_device_weights(W, b)
    Vd, Gd, W1n = _build_pencil_weights(W, b)

    # ---- device graph ----
    nc = bass.Bass()
    if N_C:
        xc_in = nc.declare_dram_parameter(
            "xallc", [N_C, 128, N_ROT, TILE_B], f16, isOutput=False
        )
        outc_ext = nc.declare_dram_parameter(
            "outc", [OUTPUT_DIM, N_C * TILE_B], f32, isOutput=True
        )
    if N_P:
        xp_in = nc.declare_dram_parameter(
            "xpen", [N_P, 128, TILE_B], f16, isOutput=False
        )
        v_in = nc.declare_dram_parameter("Vd", [128, N_PROJ], f16, isOutput=False)
        g_in = nc.declare_dram_parameter(
            "Gd", [128, N_PBLK * OUTPUT_DIM], f16, isOutput=False
        )
        w1n_in = nc.declare_dram_parameter("W1n", [128, 64], f16, isOutput=False)
        outp_ext = nc.declare_dram_parameter(
            "outp", [OUTPUT_DIM, N_P * TILE_B], f32, isOutput=True
        )
    w_in = nc.declare_dram_parameter("Wd", [128, 66 * 64], f16, isOutput=False)
    b_in = nc.declare_dram_parameter("bias", [OUTPUT_DIM, 1], f32, isOutput=False)

    # multi-class ops: one per anchor family, constant-stride rotation APs
    MC_OPS = [list(range(0, 9))] + [
        list(range(8 * k + 1, 8 * k + 9)) for k in range(1, 8)
    ]

    def rot_group_ap(xrt, classes):
        """[128, len(classes), TILE_B] APs (in0, in1)."""
        m = len(classes)
        us = [ROT_IDX[CLASS_OPS[s][0]] for s in classes]
        vs = [ROT_IDX[CLASS_OPS[s][1]] for s in classes]

        def mk(idx):
            if all(i == idx[0] for i in idx):
                return xrt[:, idx[0]: idx[0] + 1, :].to_broadcast(
                    [128, m, TILE_B]
                )
            d = idx[1] - idx[0]
            assert all(idx[j + 1] - idx[j] == d for j in range(m - 1)), idx
            return xrt[:, idx[0]:: d, :][:, 0:m, :]

        return mk(us), mk(vs)

    with tile.TileContext(nc) as tc:
        with (
            tc.tile_pool(name="consts", bufs=1) as consts,
            tc.tile_pool(name="xc", bufs=3) as xcp,
            tc.tile_pool(name="xp", bufs=2) as xpp,
            tc.tile_pool(name="prod", bufs=3) as prodp,
            tc.tile_pool(name="z2", bufs=(2 if N_C == 0 else 1)) as z2p,
            tc.tile_pool(name="outp", bufs=3) as outp,
            tc.tile_pool(name="proj", bufs=2, space="PSUM") as projp,
            tc.tile_pool(name="acc", bufs=2, space="PSUM") as accp,
        ):
            # contraction consts go on the GpSimd (SWDGE) queue; V rides
            # early on the fast sync queue (it gates the first projections)
            if N_P:
                v_sb = consts.tile([128, N_PROJ], f16)
                g_sb = consts.tile([128, N_PBLK * OUTPUT_DIM], f16)
                nc.gpsimd.dma_start(g_sb[:], g_in[:])
                w1n_sb = consts.tile([128, 64], f16)
                nc.gpsimd.dma_start(w1n_sb[:], w1n_in[:])
            w_sb = consts.tile([128, 66 * 64], f16)
            b_sb = consts.tile([OUTPUT_DIM, 1], f32)

            loaded = {}

            def load_input(idx, mode):
                if mode == "c":
                    xt = xcp.tile([128, N_ROT, TILE_B], f16, tag="xc", name="xc_t")
                    nc.sync.dma_start(xt[:], xc_in[idx][:])
                else:
                    xt = xpp.tile([128, TILE_B], f16, tag="xp", name="xp_t")
                    nc.sync.dma_start(xt[:], xp_in[idx][:])
                loaded[(mode, idx)] = xt

            def class_tile(xrt, ic):
                # acc halves: even classes + linear -> partitions 0:64,
                # odd classes -> partitions 64:128
                acc = accp.tile([128, TILE_B], f32, name="acc")
                nc.tensor.matmul(
                    acc[0:64, :],
                    w_sb[:, 0:64],
                    xrt[:, 0, :],
                    start=True,
                    stop=False,
                    tile_position=(0, 0),
                )
                first_odd = True
                for k, classes in enumerate(MC_OPS):
                    m = len(classes)
                    p_t = prodp.tile(
                        [128, m, TILE_B], f16, tag="prod" + str(m), name="p_t"
                    )
                    in0, in1 = rot_group_ap(xrt, classes)
                    nc.vector.tensor_mul(p_t[:], in0, in1)
                    for j, s in enumerate(classes):
                        half = s % 2
                        blk = 1 + s
                        nc.tensor.matmul(
                            acc[64 * half: 64 * half + 64, :],
                            w_sb[:, blk * 64: (blk + 1) * 64],
                            p_t[:, j, :],
                            start=(half == 1 and first_odd),
                            stop=(s == 64 or s == 63),
                            tile_position=(0, 64 * half),
                        )
                        if half == 1:
                            first_odd = False

                # ACT evacuates both PSUM halves; accumulating DMA adds the
                # odd half into DRAM (keeps DVE free for products)
                o_t = outp.tile([OUTPUT_DIM, TILE_B], f32, tag="o", name="o_t")
                o2_t = outp.tile([OUTPUT_DIM, TILE_B], f32, tag="o2", name="o2_t")
                nc.scalar.activation(
                    o_t[:],
                    acc[0:64, :],
                    mybir.ActivationFunctionType.Identity,
                    bias=b_sb[:, 0:1],
                )
                nc.scalar.copy(o2_t[:], acc[64:128, :])
                bs = slice(ic * TILE_B, (ic + 1) * TILE_B)
                nc.sync.dma_start(outc_ext[:, bs], o_t[:])
                nc.gpsimd.dma_start(
                    outc_ext[:, bs], o2_t[:], accum_op=mybir.AluOpType.add
                )

            def pencil_projections(x_t):
                # 48 projection matmuls -> PSUM triples -> Square -> z2 (fp16)
                z2 = z2p.tile([128, N_PBLK, TILE_B], f16, tag="z2", name="z2_t")
                for jj in range(N_PBLK // 3):
                    pj = projp.tile([128, 3, TILE_B], f32, name="pj")
                    for h in range(3):
                        j = 3 * jj + h
                        nc.tensor.matmul(
                            pj[:, h, :],
                            v_sb[:, j * 128: (j + 1) * 128],
                            x_t[:],
                            start=True,
                            stop=True,
                        )
                    dst = z2[:, 3 * jj: 3 * jj + 3, :]
                    if SQ_DVE and jj % SQ_DVE == SQ_DVE - 1:
                        nc.vector.tensor_mul(dst, pj[:], pj[:])
                    else:
                        nc.scalar.activation(
                            dst, pj[:], mybir.ActivationFunctionType.Square
                        )
                return z2

            def pencil_contraction(z2, x_t, ip):
                # dense contraction: linear + one [128K x 64N] matmul per
                # z2 block, 2-way column-tiled (even j -> psum rows 0:64,
                # odd j -> 64:128), mirroring the class-tile structure
                acc = accp.tile([128, TILE_B], f32, name="acc")
                nc.tensor.matmul(
                    acc[0:64, :], w1n_sb[:], x_t[:], start=True, stop=False,
                    tile_position=(0, 0),
                )
                for j in range(N_PBLK):
                    half = j % 2
                    nc.tensor.matmul(
                        acc[64 * half: 64 * half + 64, :],
                        g_sb[:, j * 64: (j + 1) * 64],
                        z2[:, j, :],
                        start=(j == 1),
                        stop=(j >= N_PBLK - 2),
                        tile_position=(0, 64 * half),
                    )

                o_t = outp.tile([OUTPUT_DIM, TILE_B], f32, tag="o", name="o_t")
                o2_t = outp.tile([OUTPUT_DIM, TILE_B], f32, tag="o2", name="o2_t")
                nc.scalar.activation(
                    o_t[:],
                    acc[0:64, :],
                    mybir.ActivationFunctionType.Identity,
                    bias=b_sb[:, 0:1],
                )
                nc.scalar.copy(o2_t[:], acc[64:128, :])
                bs = slice(ip * TILE_B, (ip + 1) * TILE_B)
                nc.sync.dma_start(outp_ext[:, bs], o_t[:])
                nc.gpsimd.dma_start(
                    outp_ext[:, bs], o2_t[:], accum_op=mybir.AluOpType.add
                )

            # emission schedule: 'A' = pencil projections+squares (feeds
            # ACT), 'B' = class products+matmuls then pencil contraction
            # (feeds DVE, tensor), 'c' = unpaired class tile. Unpaired
            # class tiles are interleaved between pairs so their DVE work
            # overlaps neighbouring pencil ACT work:
            #   A0 B0 A1 c A2 B1(c) ... pattern below: A_i emitted, then
            #   one B or lone-c keeps DVE fed while squares run.
            n_pair = min(N_P, N_C)
            lone_c = list(range(n_pair, N_C))
            lone_p = list(range(n_pair, N_P))
            units = []       # ('A', ip) | ('B', ip, ic) | ('c', ic) | ('p', ip)
            for i in range(n_pair):
                units.append(("A", i))
                units.append(("B", i, i))
                if lone_c and (i + 1) % 2 == 1:
                    units.append(("c", lone_c.pop(0)))
            units += [("c", i) for i in lone_c]
            for i in lone_p:
                units.append(("A", i))
                units.append(("B", i, None))

            load_seq = []    # (mode, idx) in consumption order
            for u in units:
                if u[0] == "A":
                    load_seq.append(("p", u[1]))
                elif u[0] == "B":
                    if u[2] is not None:
                        load_seq.append(("c", u[2]))
                elif u[0] == "c":
                    load_seq.append(("c", u[1]))

            # first two inputs, then the class weights, then the rest ahead
            nload = 0

            def prefetch(n):
                nonlocal nload
                while nload < min(n, len(load_seq)):
                    load_input(load_seq[nload][1], load_seq[nload][0])
                    nload += 1

            prefetch(1)
            if N_P:
                nc.sync.dma_start(v_sb[:, 0: N_PROJ // 2], v_in[:, 0: N_PROJ // 2])
                nc.sync.dma_start(v_sb[:, N_PROJ // 2:], v_in[:, N_PROJ // 2:])
            prefetch(2)
            nc.sync.dma_start(w_sb[:], w_in[:])
            nc.sync.dma_start(b_sb[:], b_in[:])
            prefetch(4)

            done = 0
            z2_of = {}
            for u in units:
                if u[0] == "A":
                    z2_of[u[1]] = pencil_projections(loaded[("p", u[1])])
                    done += 1
                elif u[0] == "B":
                    _, ip, ic = u
                    if ic is not None:
                        class_tile(loaded[("c", ic)], ic)
                        done += 1
                    pencil_contraction(z2_of.pop(ip), loaded[("p", ip)], ip)
                elif u[0] == "c":
                    class_tile(loaded[("c", u[1])], u[1])
                    done += 1
                prefetch(done + 3)

    _split_multiwaits(nc, mybir)

    # ---- per-core input maps ----
    c_tiles = [t for t in range(N_TILES) if TILE_MODE[t] == "c"]
    p_tiles = [t for t in range(N_TILES) if TILE_MODE[t] == "p"]
    in_maps = []
    for c in range(N_CORES):
        base = c * B_CORE
        m = {"Wd": w_packed, "bias": bias.reshape(OUTPUT_DIM, 1)}
        if N_C:
            xc = np.stack([
                xall[:, :, base + t * TILE_B: base + (t + 1) * TILE_B]
                for t in c_tiles
            ])  # [N_C, 128, 16, TILE_B]
            m["xallc"] = np.ascontiguousarray(xc)
        if N_P:
            xp = np.stack([
                xT[:, base + t * TILE_B: base + (t + 1) * TILE_B]
                for t in p_tiles
            ])  # [N_P, 128, TILE_B]
            m["xpen"] = np.ascontiguousarray(xp)
            m["Vd"] = Vd
            m["Gd"] = Gd
            m["W1n"] = W1n
        in_maps.append(m)
    return nc, in_maps


def kernel(x, W, b, indices_0, indices_1):
    from concourse.bass_utils import run_bass_kernel_spmd

    nc, in_maps = build(x, W, b)
    res = run_bass_kernel_spmd(nc, in_maps, list(range(N_CORES))).results

    c_tiles = [t for t in range(N_TILES) if TILE_MODE[t] == "c"]
    p_tiles = [t for t in range(N_TILES) if TILE_MODE[t] == "p"]

    out = np.empty((BATCH, OUTPUT_DIM), np.float32)
    for c in range(N_CORES):
        base = c * B_CORE
        for name, tiles in (("outc", c_tiles), ("outp", p_tiles)):
            if not tiles:
                continue
            res_t = np.asarray(res[c][name], np.float32)  # [64, n*512]
            for i, t in enumerate(tiles):
                blk = res_t[:, i * TILE_B: (i + 1) * TILE_B]
                out[base + t * TILE_B: base + (t + 1) * TILE_B] = blk.T
    return out
